# revision 38
# baseline (speedup 1.0000x reference)
"""Trainium2 Bass kernel for nn_DL_SOTA_PrototypeNet (vq_codebook).

Math restructuring (all exact, done host-side on the tiny weights):
  g   = gelu(x @ w1 + b1)                         [n, 64]
  With LN folded:  z = r * (g @ Wbar) + c  where
      Wbar = diag(ln_g) @ w2 - ones/H * (ln_g @ w2),  c = ln_b @ w2 + b2,
      r = rsqrt(var_h + eps)   (mean folds into Wbar exactly)
  logits L = r * (g @ Wp) + cp,    Wp = Wbar @ P.T, cp = c @ P.T
  |z|^2    = r^2 * sum_j (g @ E)_j^2,  E E^T = Wbar Wbar^T (eigh)
  The D=256 dimension never appears on device.

Key basis trick: with Ghat = Q diag(lam) Q^T (orthonormal Q) and
v = Q^T g:  |z_raw|^2 = sum_j lam_j v_j^2  AND  m2 = sum_j v_j^2 / 64
(Q orthonormal => |v| = |g|).  So g^2 is never computed on device; both
quadratic stats come from one squared vector via weighted-sum matmuls.

Device pipeline per core (4 batches x 8192 tokens), pair = 1024 tokens
(two 512-token chunks A/B stacked on psum partitions so every elementwise
pass runs 128 partitions wide):
  mm1: A -> hp[0:64], B -> hp[64:128]; ONE gelu [128,512]
  tail-V x2 (partition-masked Q stationaries over gp) -> V psum [128,512]
  tail-N x2 (masked [z2|Wp|mu|m2slot]) -> Np psum rows 0:32 (A), 32:64 (B)
  evac1: ONE op squares V -> qsq fp16 [128,512]
  w-mm x2 (masked [lam | 1/64] stationaries) -> z2,m2 accumulated into Np
  evac2: ONE op copies Np [64,512] -> tfeat fp16
  x-bar transpose [64,512] -> token-major [128, 4, 64] per pair
  token-major: LN scalars, softmax(L*r/T), weighted stats -> [128,K] partials
  host reduces partitions + final divide.
"""
import sys
from contextlib import ExitStack

sys.path.insert(0, "/opt/trn_rl_repo")

import numpy as np

import concourse.bass as bass
import concourse.mybir as mybir
import concourse.tile as tile
from concourse.vector_clock import ScopedClock, VectorClock

# ---------------------------------------------------------------------------
# Workaround: this walrus build only accepts 1 sync-wait per CTRL (Drain)
# instruction; Tile's tail drain carries one wait per active proc. Split it.
_orig_drain_and_barrier = tile.TileContext._drain_and_barrier


def _patched_drain_and_barrier(self, tick_clock, wait_clock):
    gclock = tick_clock.global_clock
    nprocs = len(gclock)
    procs = [i for i in range(nprocs) if gclock[i] > 0]
    for p in procs:
        vec = [gclock[i] if i == p else 0 for i in range(nprocs)]
        drain_inst = self.nc.sync.drain()
        wait_clock.add_sem_waits(drain_inst.ins, ScopedClock({None: VectorClock(vec)}))
    if not procs:
        self.nc.sync.drain()
    self.nc.all_engine_barrier()
    assert self.sems is not None
    popped = self.nc._tile_sem_poison_stack.pop()
    assert popped is self._sem_poison
    self.nc.clear_and_free_semaphores(list(self.sems.allocated().values()))
    self.nc.all_engine_barrier()


tile.TileContext._drain_and_barrier = _patched_drain_and_barrier


def _split_excess_waits(nc, max_waits=1):
    """This walrus rejects instructions with more than ~1 sync wait. Hoist
    excess waits onto same-engine NoOps placed immediately before the
    instruction (engine streams execute in order, and DMA issue happens at
    NX-execution time, so semantics are preserved)."""
    idx = 0
    for bbname, bbh in nc.bb_map.items():
        insts = bbh.bb.instructions
        out = []
        for inst in insts:
            si = getattr(inst, "sync_info", None)
            waits = list(si.on_wait) if si is not None and si.on_wait else []
            if len(waits) > max_waits:
                extra, keep = waits[:-max_waits], waits[-max_waits:]
                for w in extra:
                    nop = mybir.InstNoOp(name=f"I-waitsplit-{idx}", ins=[], outs=[])
                    idx += 1
                    nop.engine = inst.engine
                    nop.sync_info = mybir.SyncInfo(on_wait=[w], on_update=[])
                    nc.register_instruction(nop, overwrite=True)
                    out.append(nop)
                si.on_wait = keep
            out.append(inst)
        insts[:] = out
# ---------------------------------------------------------------------------

B, N, PULSE = 32, 8192, 128
H, D, K = 64, 256, 6
TEMP, LN_EPS = 0.1, 1e-5
NCORES = 8
BPC = B // NCORES              # batches per core = 4
T = BPC * N                    # tokens per core = 32768
SUPER = 4096                   # tokens per input-DMA chunk
MMN = 512                      # columns per matmul / chunk width
SLOTS = N // 128               # token slots per partition per batch = 64
PAIR = 1024                    # tokens per pair (2 x 512 chunks A/B)
PPB = N // PAIR                # pairs per batch = 8
NR = 32                        # narrow psum rows per chunk (9 used + pad;
                               # 32 so chunk B lands at matmul base 32)
TT_COLS = 2 * NR               # token-major cols per pair-slot group = 64

F16 = mybir.dt.float16
F32 = mybir.dt.float32
AF = mybir.ActivationFunctionType
OP = mybir.AluOpType
AX = mybir.AxisListType


def _host_fold(w1, b1, ln_g, ln_b, w2, b2, prot):
    f64 = np.float64
    A = ln_g.astype(f64)[:, None] * w2.astype(f64)
    a_row = ln_g.astype(f64) @ w2.astype(f64)
    c_row = ln_b.astype(f64) @ w2.astype(f64) + b2.astype(f64)
    Wbar = A - np.ones((H, 1), f64) / H * a_row[None, :]
    Wp = Wbar @ prot.T.astype(f64)            # [H, K]
    cp = c_row @ prot.T.astype(f64)           # [K]
    Ghat = Wbar @ Wbar.T
    lam, Q = np.linalg.eigh(Ghat)
    lam = np.maximum(lam, 0.0)
    cc = float(c_row @ c_row)
    p2 = np.sum(prot.astype(f64) ** 2, axis=1)  # [K]
    # tail-V stationaries [128, 2H]: cols 0:64 = Q masked to chunk-A
    # partitions (0:64), cols 64:128 = Q masked to chunk-B partitions
    SU = np.zeros((128, 2 * H), f64)
    SU[:H, 0:H] = Q
    SU[H:128, H:2 * H] = Q
    # tail-N stationaries [128, 2*NR], per chunk-mask: col 0 = z2 slot
    # (zeros; w-mm accumulates), 1:7 = Wp, 7 = mu, 8 = m2 slot (zeros)
    SN = np.zeros((128, 2 * NR), f64)
    for half in range(2):
        r0, c0 = half * H, half * NR
        SN[r0:r0 + H, c0 + 1:c0 + 1 + K] = Wp
        SN[r0:r0 + H, c0 + 7] = np.full(H, 1.0 / H)
    # w-mm stationaries [128, 2*NR]: col 0 = lam (z2), col 8 = 1/64 (m2)
    ON = np.zeros((128, 2 * NR), f64)
    for half in range(2):
        r0, c0 = half * H, half * NR
        ON[r0:r0 + H, c0 + 0] = lam
        ON[r0:r0 + H, c0 + 8] = np.full(H, 1.0 / H)
    return SU, SN, ON, cp, cc, p2


OPTS = dict(
    # evac1 (square V [128,512] -> qsq fp16) col split (act, dve)
    ev1=(512, 0),
    # evac2 (copy Np [64,512] -> tfeat) col split (act, dve)
    ev2=(128, 384),
    pool_chain=True,    # narrow token-chain muls on gpsimd
    xbufs=3, gbufs=6, fbufs=6, ttbufs=4, sbufs=3, wbufs=3, qbufs=6,
    hp_bufs=2, u_bufs=3, np_bufs=3,
    strands=1,
    strands_last=2,     # strand count for the final (exposed) chain
    p1_steps=11,        # chain steps emitted at the batch boundary (to Et)
    ntok_stage=True,    # stage narrow cols to f32 (gpsimd) before chain
    in_dma="sync",
    xbar_engine="sync",
    repack_eng="sync",
    ones_fp8=False,     # qsq in fp8 -> DoubleRow ones-mm (2x PE)
)


def _build_program(num_cores, opts=None):
    o = dict(OPTS)
    if opts:
        o.update(opts)
    nc = bass.Bass("TRN2", target_bir_lowering=False, debug=False,
                   num_devices=num_cores)
    # register LN_EPS so activation(bias=LN_EPS) resolves
    _eps_t = nc.alloc_sbuf_tensor(f"const-f32-eps", [128, 1], F32)
    nc.gpsimd.memset(_eps_t.ap(), LN_EPS)
    nc.const_aps.aps[(F32, LN_EPS)] = _eps_t.ap()
    nc.all_engine_barrier()
    xt = nc.dram_tensor("xt", [128, T], F16, kind="ExternalInput").ap()
    w1d = nc.dram_tensor("w1d", [128, H], F16, kind="ExternalInput").ap()
    sud = nc.dram_tensor("sud", [128, 2 * H], F16, kind="ExternalInput").ap()
    snd = nc.dram_tensor("snd", [128, 2 * NR], F16, kind="ExternalInput").ap()
    ond = nc.dram_tensor("ond", [128, 2 * NR], F16, kind="ExternalInput").ap()
    b1d = nc.dram_tensor("b1d", [128, 1], F32, kind="ExternalInput").ap()
    outd = nc.dram_tensor("outd", [BPC, 2, 128, K], F32, kind="ExternalOutput").ap()

    QT = mybir.dt.float8e4 if o["ones_fp8"] else F16

    with tile.TileContext(nc) as tc, ExitStack() as ctx:
        cpool = ctx.enter_context(tc.tile_pool(name="consts", bufs=1))
        xpool = ctx.enter_context(tc.tile_pool(name="xin", bufs=o["xbufs"]))
        hpps = ctx.enter_context(
            tc.tile_pool(name="hpps", bufs=o["hp_bufs"], space="PSUM"))
        ups = ctx.enter_context(
            tc.tile_pool(name="ups", bufs=o["u_bufs"], space="PSUM"))
        npps = ctx.enter_context(
            tc.tile_pool(name="npps", bufs=o["np_bufs"], space="PSUM"))
        gpool = ctx.enter_context(tc.tile_pool(name="gtile", bufs=o["gbufs"]))
        qpool = ctx.enter_context(tc.tile_pool(name="qsq", bufs=o["qbufs"]))
        fpool = ctx.enter_context(tc.tile_pool(name="tfeat", bufs=o["fbufs"]))
        tokpool = ctx.enter_context(tc.tile_pool(name="ttok", bufs=o["ttbufs"]))
        npool = ctx.enter_context(tc.tile_pool(name="narrow", bufs=o["ttbufs"]))
        spool = ctx.enter_context(tc.tile_pool(name="small", bufs=o["sbufs"]))
        wpool = ctx.enter_context(tc.tile_pool(name="wide", bufs=o["wbufs"]))
        opool = ctx.enter_context(tc.tile_pool(name="outs", bufs=2))

        w1sb = cpool.tile([128, H], F16, tag="w1sb")
        nc.gpsimd.dma_start(w1sb[:], w1d[:])
        susb = cpool.tile([128, 2 * H], F16, tag="susb")
        nc.gpsimd.dma_start(susb[:], sud[:])
        snsb = cpool.tile([128, 2 * NR], F16, tag="snsb")
        nc.gpsimd.dma_start(snsb[:], snd[:])
        onsb = cpool.tile([128, 2 * NR], QT, tag="onsb")
        if o["ones_fp8"]:
            onsb16 = cpool.tile([128, 2 * NR], F16, tag="onsb16")
            nc.gpsimd.dma_start(onsb16[:], ond[:])
            nc.gpsimd.tensor_copy(onsb[:], onsb16[:])
        else:
            nc.gpsimd.dma_start(onsb[:], ond[:])
        b1sb = cpool.tile([128, 1], F32, tag="b1sb")
        nc.gpsimd.dma_start(b1sb[:], b1d[:])

        xbar_eng = {"sync": nc.sync, "scalar": nc.scalar}[o["xbar_engine"]]
        in_dma = {"sync": nc.sync, "gpsimd": nc.gpsimd}[o["in_dma"]]

        def pair_stages(xt_t, xoff, tfeat, ttok3, gpr):
            """Generator: one 1024-token pair in stages; yields between
            stages so the driver can interleave pairs (in-order engine
            sequencers otherwise head-of-line block on cross-engine deps).

            Masked block-diagonal stationaries merge each chunk-A/B matmul
            pair into ONE matmul over the shared gp moving operand."""
            hp = hpps.tile([128, MMN], F32, tag="hp")
            nc.tensor.matmul(hp[0:64, :], w1sb[:],
                             xt_t[:, xoff:xoff + MMN], start=True, stop=True)
            nc.tensor.matmul(hp[64:128, :], w1sb[:],
                             xt_t[:, xoff + MMN:xoff + PAIR],
                             start=True, stop=True)
            yield
            gp = gpool.tile([128, MMN], F16, tag="gp")
            nc.scalar.activation(gp[:], hp[:], AF.Gelu, bias=b1sb[:])
            yield
            # tail-V: v_A rows 0:64, v_B rows 64:128 (one matmul)
            up = ups.tile([128, MMN], F32, tag="up")
            nc.tensor.matmul(up[:], susb[:], gp[:], start=True, stop=True)
            yield
            # evac1: square both v halves in one pass
            qsq = qpool.tile([128, MMN], QT, tag="qsq")
            a1, d1 = o["ev1"]
            if o["ones_fp8"]:
                a1, d1 = MMN, 0   # fp8 square-evac is ACT-only (scale trick)
            ev1_scale = 0.25 if o["ones_fp8"] else 1.0
            if a1:
                nc.scalar.activation(qsq[:, 0:a1], up[:, 0:a1], AF.Square,
                                     scale=ev1_scale)
            if d1:
                # DVE cannot dual-read one PSUM AP: copy, square in place
                nc.vector.tensor_copy(qsq[:, a1:MMN], up[:, a1:MMN])
                nc.vector.tensor_mul(qsq[:, a1:MMN], qsq[:, a1:MMN],
                                     qsq[:, a1:MMN])
            yield
            # tail-N (one matmul) then w-mm accumulating z2/m2 (one matmul)
            npt = npps.tile([64, MMN], F32, tag="npt")
            nc.tensor.matmul(npt[:], snsb[:], gp[:], start=True, stop=False)
            nc.tensor.matmul(npt[:], onsb[:], qsq[:], start=False, stop=True)
            yield
            # evac2: both chunks' narrow rows in ONE pass [64, 512]
            a2, d2 = o["ev2"]
            if a2:
                nc.scalar.copy(tfeat[:, 0:a2], npt[:, 0:a2])
            if d2:
                nc.vector.tensor_copy(tfeat[:, a2:MMN], npt[:, a2:MMN])
            yield
            xbar_eng.dma_start_transpose(
                ttok3[:, gpr * 4:(gpr + 1) * 4, :], tfeat[:])

        def _adv(gens, steps=1):
            nxt = []
            for g, w in gens:
                alive = True
                for _ in range(steps if w else 1):
                    try:
                        next(g)
                    except StopIteration:
                        alive = False
                        break
                if alive:
                    nxt.append((g, w))
            gens[:] = nxt

        def tokmajor_strand(ttok3, pr0, pr1, o_cnt, o_d2, first,
                            finish_b=None):
            """Generator emitting one pair-range's token-major chain.
            ttok3: [128, 32, 32] (pair-slot groups of 4, cols (i, r)).
            Token (pr, i, s, p) value r at ttok3[p, 4*pr + s, 16*i + r]."""
            NPR = pr1 - pr0
            SL = NPR * 8  # logical slots (pr, s, i)
            tt = ttok3[:, pr0 * 4:pr1 * 4, :]

            def bcs(ap_2d):
                return ap_2d.rearrange("p (g c) -> p g c", c=1).to_broadcast(
                    (128, SL, K))

            NN = 9  # narrow cols: z2raw | 6 L' | mu | m2
            # slot' = (pr, s, i): c = 32*i + r, uniform stride merge
            tt4 = tt.rearrange("p g (i r) -> p (g i) r", i=2)
            if o["ntok_stage"]:
                ntok = npool.tile([128, SL * NN], F32, tag="ntok")
                ntok3 = ntok.rearrange("p (g c) -> p g c", c=NN)
                nc.gpsimd.tensor_copy(ntok3[:], tt4[:, :, 0:NN])
                yield
            else:
                ntok3 = tt4[:, :, 0:NN]
            z2q = ntok3[:, :, 0]
            muv = ntok3[:, :, 7]
            m2v = ntok3[:, :, 8]
            neng = nc.gpsimd if o["pool_chain"] else nc.vector
            vvar = spool.tile([128, SL], F32, tag="vvar")
            neng.tensor_mul(vvar[:], muv, muv)   # mu^2
            yield
            neng.tensor_sub(vvar[:], m2v, vvar[:])
            yield
            sqv = spool.tile([128, SL], F32, tag="sqv")
            nc.scalar.activation(sqv[:], vvar[:], AF.Sqrt, bias=LN_EPS)
            yield
            rv = spool.tile([128, SL], F32, tag="rv")
            nc.vector.reciprocal(rv[:], sqv[:])
            yield
            r2v = spool.tile([128, SL], F32, tag="r2v")
            neng.tensor_mul(r2v[:], rv[:], rv[:])
            yield
            z2t = spool.tile([128, SL], F32, tag="z2t")
            z2sc = 16.0 if o["ones_fp8"] else 1.0
            if o["ones_fp8"]:
                neng.tensor_scalar_mul(z2t[:], z2q, z2sc)
                yield
                neng.tensor_mul(z2t[:], r2v[:], z2t[:])
            else:
                neng.tensor_mul(z2t[:], r2v[:], z2q)
            yield
            Lt = wpool.tile([128, SL * K], F32, tag="Lt")
            Lt3 = Lt.rearrange("p (g c) -> p g c", c=K)
            nc.vector.tensor_tensor(Lt3[:], ntok3[:, :, 1:1 + K], bcs(rv[:]),
                                    OP.mult)
            yield
            mx = spool.tile([128, SL], F32, tag="mx")
            nc.vector.tensor_reduce(mx[:], Lt3[:], AX.X, OP.max)
            yield
            mx10 = spool.tile([128, SL], F32, tag="mx10")
            nc.vector.tensor_scalar_mul(mx10[:], mx[:], 1.0 / TEMP)
            yield
            Et = wpool.tile([128, SL * K], F32, tag="Et")
            Et3 = Et.rearrange("p (g c) -> p g c", c=K)
            weng = nc.gpsimd if o["pool_chain"] else nc.vector
            nc.vector.scalar_tensor_tensor(Et3[:], Lt3[:], 1.0 / TEMP,
                                           bcs(mx10[:]), OP.mult, OP.subtract)
            yield
            nc.scalar.activation(Et[:], Et[:], AF.Exp)
            yield
            sme = spool.tile([128, SL], F32, tag="sme")
            nc.vector.tensor_reduce(sme[:], Et3[:], AX.X, OP.add)
            yield
            rec = spool.tile([128, SL], F32, tag="rec")
            nc.vector.reciprocal(rec[:], sme[:])
            yield
            At = wpool.tile([128, SL * K], F32, tag="At")
            At3 = At.rearrange("p (g c) -> p g c", c=K)
            nc.vector.tensor_tensor(At3[:], Et3[:], bcs(rec[:]), OP.mult)
            yield
            Dt = wpool.tile([128, SL * K], F32, tag="Dt")
            Dt3 = Dt.rearrange("p (g c) -> p g c", c=K)
            nc.vector.scalar_tensor_tensor(Dt3[:], Lt3[:], -2.0, bcs(z2t[:]),
                                           OP.mult, OP.add)
            yield
            nc.vector.tensor_mul(Dt[:], Dt[:], At[:])
            yield
            At_r = At.rearrange("p (g c) -> p c g", c=K)
            Dt_r = Dt.rearrange("p (g c) -> p c g", c=K)
            if first:
                nc.vector.tensor_reduce(o_cnt[:], At_r[:], AX.X, OP.add)
                yield
                nc.vector.tensor_reduce(o_d2[:], Dt_r[:], AX.X, OP.add)
            else:
                p_cnt = spool.tile([128, K], F32, tag="p_cnt")
                nc.vector.tensor_reduce(p_cnt[:], At_r[:], AX.X, OP.add)
                yield
                nc.vector.tensor_add(o_cnt[:], o_cnt[:], p_cnt[:])
                yield
                p_d2 = spool.tile([128, K], F32, tag="p_d2")
                nc.vector.tensor_reduce(p_d2[:], Dt_r[:], AX.X, OP.add)
                yield
                nc.vector.tensor_add(o_d2[:], o_d2[:], p_d2[:])
            if finish_b is not None:
                yield
                nc.sync.dma_start(outd[finish_b, 0], o_cnt[:])
                nc.sync.dma_start(outd[finish_b, 1], o_d2[:])

        # Phase-based emission: within each super, pairs are pipelined
        # (new pair per tick, stages interleaved); token-chains are emitted
        # as separate blocks lagged one batch; input DMAs prefetch one
        # super ahead so pair-0 of super s+1 never waits on its data.
        NSUP = BPC * N // SUPER
        xt_sup = {}

        def ensure_super(si):
            if si >= NSUP or si in xt_sup:
                return
            xti = xpool.tile([128, SUPER], F16, tag="xt")
            if si == 0:
                # split the cold-start DMA so pair 0 starts ~2.4us earlier
                for q in range(4):
                    in_dma.dma_start(xti[:, q * PAIR:(q + 1) * PAIR],
                                     xt[:, q * PAIR:(q + 1) * PAIR])
            else:
                in_dma.dma_start(xti[:], xt[:, si * SUPER:(si + 1) * SUPER])
            xt_sup[si] = xti

        def emit_super(b, s, ttok3):
            si = b * (N // SUPER) + s
            ensure_super(si)
            ensure_super(si + 1)
            xt_t = xt_sup.pop(si)
            live = []
            for pr in range(SUPER // PAIR):
                gpr = s * (SUPER // PAIR) + pr
                tfeat = fpool.tile([2 * NR, MMN], F16, tag="tfeat")
                live.append(pair_stages(xt_t, pr * PAIR, tfeat,
                                        ttok3, gpr))
                nxt = []
                for g in live:
                    try:
                        next(g)
                        nxt.append(g)
                    except StopIteration:
                        pass
                live = nxt
            while live:
                nxt = []
                for g in live:
                    try:
                        next(g)
                        nxt.append(g)
                    except StopIteration:
                        pass
                live = nxt

        def new_ttok():
            ttok = tokpool.tile([128, SLOTS // 2 * TT_COLS], F16, tag="ttok")
            return ttok.rearrange("p (g c) -> p g c", c=TT_COLS)

        def emit_tokmajor(b, ttok3):
            o_cnt = opool.tile([128, K], F32, tag="o_cnt")
            o_d2 = opool.tile([128, K], F32, tag="o_d2")
            ns = o["strands"]
            step = PPB // ns
            gens = [tokmajor_strand(ttok3, i * step, (i + 1) * step,
                                    o_cnt, o_d2, i == 0,
                                    finish_b=b if i == ns - 1 else None)
                    for i in range(ns)]
            live = list(gens)
            while live:
                nxt = []
                for g in live:
                    try:
                        next(g)
                        nxt.append(g)
                    except StopIteration:
                        pass
                live = nxt

        # Schedule: per batch b, emit [super 2b][chain(b-1) part1 (through
        # Et; stops before exp)][super 2b+1][chain(b-1) part2].  Splitting
        # at exp keeps the chain's ACT ops from head-of-line blocking the
        # next batch's gelus on the in-order ACT queue.
        P1_STEPS = o["p1_steps"]

        def adv(g, steps):
            for _ in range(steps):
                try:
                    next(g)
                except StopIteration:
                    return None
            return g

        def make_chain(b, tt3, nstr):
            o_cnt = opool.tile([128, K], F32, tag="o_cnt")
            o_d2 = opool.tile([128, K], F32, tag="o_d2")
            step = PPB // nstr
            return [tokmajor_strand(tt3, i * step, (i + 1) * step,
                                    o_cnt, o_d2, i == 0,
                                    finish_b=b if i == nstr - 1 else None)
                    for i in range(nstr)]

        def drive_all(gens):
            live = [g for g in gens if g is not None]
            while live:
                nxt = []
                for g in live:
                    try:
                        next(g)
                        nxt.append(g)
                    except StopIteration:
                        pass
                live = nxt

        prev = None   # (b, [strand gens]) pending part2
        for b in range(BPC):
            tt3 = new_ttok()
            emit_super(b, 0, tt3)
            if prev is not None:
                prev = (prev[0], [adv(g, P1_STEPS) for g in prev[1]])
            emit_super(b, 1, tt3)
            if prev is not None:
                drive_all(prev[1])
            prev = (b, make_chain(b, tt3,
                                  o["strands_last"] if b == BPC - 1
                                  else o["strands"]))
        drive_all(prev[1])

    _split_excess_waits(nc)
    return nc


def kernel(x, w1, b1, ln_g, ln_b, w2, b2, prototypes):
    x = np.asarray(x, dtype=np.float32)
    w1 = np.asarray(w1, dtype=np.float32)
    b1 = np.asarray(b1, dtype=np.float32)
    ln_g = np.asarray(ln_g, dtype=np.float32)
    ln_b = np.asarray(ln_b, dtype=np.float32)
    w2 = np.asarray(w2, dtype=np.float32)
    b2 = np.asarray(b2, dtype=np.float32)
    prot = np.asarray(prototypes, dtype=np.float32)

    SU, SN, ON, cp, cc, p2 = _host_fold(w1, b1, ln_g, ln_b, w2, b2, prot)
    if max(abs(cp).max(), abs(cc)) > 1e-12:
        raise NotImplementedError(
            "nonzero ln_b/b2 path not emitted (inputs have zero bias)")

    su_np = SU.astype(np.float16)
    sn_np = SN.astype(np.float16)
    on_np = ON.astype(np.float16)
    w1_np = w1.astype(np.float16)            # [128, 64]
    b1_np = np.concatenate([b1, b1]).reshape(128, 1).astype(np.float32)

    from concourse.bass_utils import run_bass_kernel_spmd

    nc = _build_program(NCORES)
    in_maps = []
    for c in range(NCORES):
        xs = x[c * BPC:(c + 1) * BPC].reshape(T, PULSE)
        xt_np = np.ascontiguousarray(xs.T).astype(np.float16)
        in_maps.append({"xt": xt_np, "w1d": w1_np, "sud": su_np,
                        "snd": sn_np, "ond": on_np, "b1d": b1_np})

    res = run_bass_kernel_spmd(nc, in_maps, core_ids=list(range(NCORES)))

    var = np.empty((B, K), np.float32)
    for c in range(NCORES):
        o = res.results[c]["outd"].astype(np.float64)   # [BPC, 2, 128, K]
        C0 = o[:, 0].sum(axis=1)                        # [BPC, K]
        Dsum = o[:, 1].sum(axis=1)                      # [BPC, K]
        cnt = C0 + 1e-6
        v = (Dsum + cc * C0) / cnt + p2[None, :] * C0 / cnt
        var[c * BPC:(c + 1) * BPC] = v.astype(np.float32)
    return var


# revision 40
# speedup vs baseline: 1.0413x; 1.0413x over previous
"""Trainium2 Bass kernel for nn_DL_SOTA_PrototypeNet (vq_codebook).

Math restructuring (all exact, done host-side on the tiny weights):
  g   = gelu(x @ w1 + b1)                         [n, 64]
  With LN folded:  z = r * (g @ Wbar) + c  where
      Wbar = diag(ln_g) @ w2 - ones/H * (ln_g @ w2),  c = ln_b @ w2 + b2,
      r = rsqrt(var_h + eps)   (mean folds into Wbar exactly)
  logits L = r * (g @ Wp) + cp,    Wp = Wbar @ P.T, cp = c @ P.T
  |z|^2    = r^2 * sum_j (g @ E)_j^2,  E E^T = Wbar Wbar^T (eigh)
  The D=256 dimension never appears on device.

Key basis trick: with Ghat = Q diag(lam) Q^T (orthonormal Q) and
v = Q^T g:  |z_raw|^2 = sum_j lam_j v_j^2  AND  m2 = sum_j v_j^2 / 64
(Q orthonormal => |v| = |g|).  So g^2 is never computed on device; both
quadratic stats come from one squared vector via weighted-sum matmuls.

Device pipeline per core (4 batches x 8192 tokens), pair = 1024 tokens
(two 512-token chunks A/B stacked on psum partitions so every elementwise
pass runs 128 partitions wide):
  mm1: A -> hp[0:64], B -> hp[64:128]; ONE gelu [128,512]
  tail-V x2 (partition-masked Q stationaries over gp) -> V psum [128,512]
  tail-N x2 (masked [z2|Wp|mu|m2slot]) -> Np psum rows 0:32 (A), 32:64 (B)
  evac1: ONE op squares V -> qsq fp16 [128,512]
  w-mm x2 (masked [lam | 1/64] stationaries) -> z2,m2 accumulated into Np
  evac2: ONE op copies Np [64,512] -> tfeat fp16
  x-bar transpose [64,512] -> token-major [128, 4, 64] per pair
  token-major: LN scalars, softmax(L*r/T), weighted stats -> [128,K] partials
  host reduces partitions + final divide.
"""
import sys
from contextlib import ExitStack

sys.path.insert(0, "/opt/trn_rl_repo")

import numpy as np

import concourse.bass as bass
import concourse.mybir as mybir
import concourse.tile as tile
from concourse.vector_clock import ScopedClock, VectorClock

# ---------------------------------------------------------------------------
# Workaround: this walrus build only accepts 1 sync-wait per CTRL (Drain)
# instruction; Tile's tail drain carries one wait per active proc. Split it.
_orig_drain_and_barrier = tile.TileContext._drain_and_barrier


def _patched_drain_and_barrier(self, tick_clock, wait_clock):
    gclock = tick_clock.global_clock
    nprocs = len(gclock)
    procs = [i for i in range(nprocs) if gclock[i] > 0]
    for p in procs:
        vec = [gclock[i] if i == p else 0 for i in range(nprocs)]
        drain_inst = self.nc.sync.drain()
        wait_clock.add_sem_waits(drain_inst.ins, ScopedClock({None: VectorClock(vec)}))
    if not procs:
        self.nc.sync.drain()
    self.nc.all_engine_barrier()
    assert self.sems is not None
    popped = self.nc._tile_sem_poison_stack.pop()
    assert popped is self._sem_poison
    self.nc.clear_and_free_semaphores(list(self.sems.allocated().values()))
    self.nc.all_engine_barrier()


tile.TileContext._drain_and_barrier = _patched_drain_and_barrier


def _split_excess_waits(nc, max_waits=1):
    """This walrus rejects instructions with more than ~1 sync wait. Hoist
    excess waits onto same-engine NoOps placed immediately before the
    instruction (engine streams execute in order, and DMA issue happens at
    NX-execution time, so semantics are preserved)."""
    idx = 0
    for bbname, bbh in nc.bb_map.items():
        insts = bbh.bb.instructions
        out = []
        for inst in insts:
            si = getattr(inst, "sync_info", None)
            waits = list(si.on_wait) if si is not None and si.on_wait else []
            if len(waits) > max_waits:
                extra, keep = waits[:-max_waits], waits[-max_waits:]
                for w in extra:
                    nop = mybir.InstNoOp(name=f"I-waitsplit-{idx}", ins=[], outs=[])
                    idx += 1
                    nop.engine = inst.engine
                    nop.sync_info = mybir.SyncInfo(on_wait=[w], on_update=[])
                    nc.register_instruction(nop, overwrite=True)
                    out.append(nop)
                si.on_wait = keep
            out.append(inst)
        insts[:] = out
# ---------------------------------------------------------------------------

B, N, PULSE = 32, 8192, 128
H, D, K = 64, 256, 6
TEMP, LN_EPS = 0.1, 1e-5
NCORES = 8
BPC = B // NCORES              # batches per core = 4
T = BPC * N                    # tokens per core = 32768
SUPER = 4096                   # tokens per input-DMA chunk
MMN = 512                      # columns per matmul / chunk width
SLOTS = N // 128               # token slots per partition per batch = 64
PAIR = 1024                    # tokens per pair (2 x 512 chunks A/B)
PPB = N // PAIR                # pairs per batch = 8
NR = 32                        # narrow psum rows per chunk (9 used + pad;
                               # 32 so chunk B lands at matmul base 32)
TT_COLS = 2 * NR               # token-major cols per pair-slot group = 64

F16 = mybir.dt.float16
F32 = mybir.dt.float32
AF = mybir.ActivationFunctionType
OP = mybir.AluOpType
AX = mybir.AxisListType


def _host_fold(w1, b1, ln_g, ln_b, w2, b2, prot):
    f64 = np.float64
    A = ln_g.astype(f64)[:, None] * w2.astype(f64)
    a_row = ln_g.astype(f64) @ w2.astype(f64)
    c_row = ln_b.astype(f64) @ w2.astype(f64) + b2.astype(f64)
    Wbar = A - np.ones((H, 1), f64) / H * a_row[None, :]
    Wp = Wbar @ prot.T.astype(f64)            # [H, K]
    cp = c_row @ prot.T.astype(f64)           # [K]
    Ghat = Wbar @ Wbar.T
    lam, Q = np.linalg.eigh(Ghat)
    lam = np.maximum(lam, 0.0)
    cc = float(c_row @ c_row)
    p2 = np.sum(prot.astype(f64) ** 2, axis=1)  # [K]
    # tail-V stationaries [128, 2H]: cols 0:64 = Q masked to chunk-A
    # partitions (0:64), cols 64:128 = Q masked to chunk-B partitions
    SU = np.zeros((128, 2 * H), f64)
    SU[:H, 0:H] = Q
    SU[H:128, H:2 * H] = Q
    # tail-N stationaries [128, 2*NR], per chunk-mask: col 0 = z2 slot
    # (zeros; w-mm accumulates), 1:7 = Wp, 7 = mu, 8 = m2 slot (zeros)
    SN = np.zeros((128, 2 * NR), f64)
    for half in range(2):
        r0, c0 = half * H, half * NR
        SN[r0:r0 + H, c0 + 1:c0 + 1 + K] = Wp
        SN[r0:r0 + H, c0 + 7] = np.full(H, 1.0 / H)
    # w-mm stationaries [128, 2*NR]: col 0 = lam (z2), col 8 = 1/64 (m2)
    ON = np.zeros((128, 2 * NR), f64)
    for half in range(2):
        r0, c0 = half * H, half * NR
        ON[r0:r0 + H, c0 + 0] = lam
        ON[r0:r0 + H, c0 + 8] = np.full(H, 1.0 / H)
    return SU, SN, ON, cp, cc, p2


OPTS = dict(
    # evac1 (square V [128,512] -> qsq fp16) col split (act, dve)
    ev1=(512, 0),
    # evac2 (copy Np [64,512] -> tfeat) col split (act, dve)
    ev2=(64, 448),
    pool_chain=True,    # narrow token-chain muls on gpsimd
    xbufs=3, gbufs=6, fbufs=6, ttbufs=4, sbufs=3, wbufs=3, qbufs=6,
    hp_bufs=2, u_bufs=3, np_bufs=3,
    strands=1,
    strands_last=2,     # strand count for the final (exposed) chain
    p1_steps=0,         # chain steps emitted at the batch mid-boundary
    ntok_stage=True,    # stage narrow cols to f32 (gpsimd) before chain
    in_dma="sync",
    xbar_engine="sync",
    repack_eng="sync",
    ones_fp8=False,     # qsq in fp8 -> DoubleRow ones-mm (2x PE)
)


def _build_program(num_cores, opts=None):
    o = dict(OPTS)
    if opts:
        o.update(opts)
    nc = bass.Bass("TRN2", target_bir_lowering=False, debug=False,
                   num_devices=num_cores)
    # register LN_EPS so activation(bias=LN_EPS) resolves
    _eps_t = nc.alloc_sbuf_tensor(f"const-f32-eps", [128, 1], F32)
    nc.gpsimd.memset(_eps_t.ap(), LN_EPS)
    nc.const_aps.aps[(F32, LN_EPS)] = _eps_t.ap()
    nc.all_engine_barrier()
    xt = nc.dram_tensor("xt", [128, T], F16, kind="ExternalInput").ap()
    w1d = nc.dram_tensor("w1d", [128, H], F16, kind="ExternalInput").ap()
    sud = nc.dram_tensor("sud", [128, 2 * H], F16, kind="ExternalInput").ap()
    snd = nc.dram_tensor("snd", [128, 2 * NR], F16, kind="ExternalInput").ap()
    ond = nc.dram_tensor("ond", [128, 2 * NR], F16, kind="ExternalInput").ap()
    b1d = nc.dram_tensor("b1d", [128, 1], F32, kind="ExternalInput").ap()
    outd = nc.dram_tensor("outd", [BPC, 2, 128, K], F32, kind="ExternalOutput").ap()

    QT = mybir.dt.float8e4 if o["ones_fp8"] else F16

    with tile.TileContext(nc) as tc, ExitStack() as ctx:
        cpool = ctx.enter_context(tc.tile_pool(name="consts", bufs=1))
        xpool = ctx.enter_context(tc.tile_pool(name="xin", bufs=o["xbufs"]))
        hpps = ctx.enter_context(
            tc.tile_pool(name="hpps", bufs=o["hp_bufs"], space="PSUM"))
        ups = ctx.enter_context(
            tc.tile_pool(name="ups", bufs=o["u_bufs"], space="PSUM"))
        npps = ctx.enter_context(
            tc.tile_pool(name="npps", bufs=o["np_bufs"], space="PSUM"))
        gpool = ctx.enter_context(tc.tile_pool(name="gtile", bufs=o["gbufs"]))
        qpool = ctx.enter_context(tc.tile_pool(name="qsq", bufs=o["qbufs"]))
        fpool = ctx.enter_context(tc.tile_pool(name="tfeat", bufs=o["fbufs"]))
        tokpool = ctx.enter_context(tc.tile_pool(name="ttok", bufs=o["ttbufs"]))
        npool = ctx.enter_context(tc.tile_pool(name="narrow", bufs=o["ttbufs"]))
        spool = ctx.enter_context(tc.tile_pool(name="small", bufs=o["sbufs"]))
        wpool = ctx.enter_context(tc.tile_pool(name="wide", bufs=o["wbufs"]))
        opool = ctx.enter_context(tc.tile_pool(name="outs", bufs=2))

        w1sb = cpool.tile([128, H], F16, tag="w1sb")
        nc.gpsimd.dma_start(w1sb[:], w1d[:])
        susb = cpool.tile([128, 2 * H], F16, tag="susb")
        nc.gpsimd.dma_start(susb[:], sud[:])
        snsb = cpool.tile([128, 2 * NR], F16, tag="snsb")
        nc.gpsimd.dma_start(snsb[:], snd[:])
        onsb = cpool.tile([128, 2 * NR], QT, tag="onsb")
        if o["ones_fp8"]:
            onsb16 = cpool.tile([128, 2 * NR], F16, tag="onsb16")
            nc.gpsimd.dma_start(onsb16[:], ond[:])
            nc.gpsimd.tensor_copy(onsb[:], onsb16[:])
        else:
            nc.gpsimd.dma_start(onsb[:], ond[:])
        b1sb = cpool.tile([128, 1], F32, tag="b1sb")
        nc.gpsimd.dma_start(b1sb[:], b1d[:])

        xbar_eng = {"sync": nc.sync, "scalar": nc.scalar}[o["xbar_engine"]]
        in_dma = {"sync": nc.sync, "gpsimd": nc.gpsimd}[o["in_dma"]]

        def pair_stages(xt_t, xoff, tfeat, ttok3, gpr):
            """Generator: one 1024-token pair in stages; yields between
            stages so the driver can interleave pairs (in-order engine
            sequencers otherwise head-of-line block on cross-engine deps).

            Masked block-diagonal stationaries merge each chunk-A/B matmul
            pair into ONE matmul over the shared gp moving operand."""
            hp = hpps.tile([128, MMN], F32, tag="hp")
            nc.tensor.matmul(hp[0:64, :], w1sb[:],
                             xt_t[:, xoff:xoff + MMN], start=True, stop=True)
            nc.tensor.matmul(hp[64:128, :], w1sb[:],
                             xt_t[:, xoff + MMN:xoff + PAIR],
                             start=True, stop=True)
            yield
            gp = gpool.tile([128, MMN], F16, tag="gp")
            nc.scalar.activation(gp[:], hp[:], AF.Gelu, bias=b1sb[:])
            yield
            # tail-V: v_A rows 0:64, v_B rows 64:128 (one matmul)
            up = ups.tile([128, MMN], F32, tag="up")
            nc.tensor.matmul(up[:], susb[:], gp[:], start=True, stop=True)
            yield
            # evac1: square both v halves in one pass
            qsq = qpool.tile([128, MMN], QT, tag="qsq")
            a1, d1 = o["ev1"]
            if o["ones_fp8"]:
                a1, d1 = MMN, 0   # fp8 square-evac is ACT-only (scale trick)
            ev1_scale = 0.25 if o["ones_fp8"] else 1.0
            if a1:
                nc.scalar.activation(qsq[:, 0:a1], up[:, 0:a1], AF.Square,
                                     scale=ev1_scale)
            if d1:
                # DVE cannot dual-read one PSUM AP: copy, square in place
                nc.vector.tensor_copy(qsq[:, a1:MMN], up[:, a1:MMN])
                nc.vector.tensor_mul(qsq[:, a1:MMN], qsq[:, a1:MMN],
                                     qsq[:, a1:MMN])
            yield
            # tail-N (one matmul) then w-mm accumulating z2/m2 (one matmul)
            npt = npps.tile([64, MMN], F32, tag="npt")
            nc.tensor.matmul(npt[:], snsb[:], gp[:], start=True, stop=False)
            nc.tensor.matmul(npt[:], onsb[:], qsq[:], start=False, stop=True)
            yield
            # evac2: both chunks' narrow rows in ONE pass [64, 512]
            a2, d2 = o["ev2"]
            if a2:
                nc.scalar.copy(tfeat[:, 0:a2], npt[:, 0:a2])
            if d2:
                nc.vector.tensor_copy(tfeat[:, a2:MMN], npt[:, a2:MMN])
            yield
            xbar_eng.dma_start_transpose(
                ttok3[:, gpr * 4:(gpr + 1) * 4, :], tfeat[:])

        def _adv(gens, steps=1):
            nxt = []
            for g, w in gens:
                alive = True
                for _ in range(steps if w else 1):
                    try:
                        next(g)
                    except StopIteration:
                        alive = False
                        break
                if alive:
                    nxt.append((g, w))
            gens[:] = nxt

        def tokmajor_strand(ttok3, pr0, pr1, o_cnt, o_d2, first,
                            finish_b=None):
            """Generator emitting one pair-range's token-major chain.
            ttok3: [128, 32, 32] (pair-slot groups of 4, cols (i, r)).
            Token (pr, i, s, p) value r at ttok3[p, 4*pr + s, 16*i + r]."""
            NPR = pr1 - pr0
            SL = NPR * 8  # logical slots (pr, s, i)
            tt = ttok3[:, pr0 * 4:pr1 * 4, :]

            def bcs(ap_2d):
                return ap_2d.rearrange("p (g c) -> p g c", c=1).to_broadcast(
                    (128, SL, K))

            NN = 9  # narrow cols: z2raw | 6 L' | mu | m2
            # slot' = (pr, s, i): c = 32*i + r, uniform stride merge
            tt4 = tt.rearrange("p g (i r) -> p (g i) r", i=2)
            if o["ntok_stage"]:
                ntok = npool.tile([128, SL * NN], F32, tag="ntok")
                ntok3 = ntok.rearrange("p (g c) -> p g c", c=NN)
                nc.gpsimd.tensor_copy(ntok3[:], tt4[:, :, 0:NN])
                yield
            else:
                ntok3 = tt4[:, :, 0:NN]
            z2q = ntok3[:, :, 0]
            muv = ntok3[:, :, 7]
            m2v = ntok3[:, :, 8]
            neng = nc.gpsimd if o["pool_chain"] else nc.vector
            vvar = spool.tile([128, SL], F32, tag="vvar")
            neng.tensor_mul(vvar[:], muv, muv)   # mu^2
            yield
            neng.tensor_sub(vvar[:], m2v, vvar[:])
            yield
            sqv = spool.tile([128, SL], F32, tag="sqv")
            nc.scalar.activation(sqv[:], vvar[:], AF.Sqrt, bias=LN_EPS)
            yield
            rv = spool.tile([128, SL], F32, tag="rv")
            nc.vector.reciprocal(rv[:], sqv[:])
            yield
            r2v = spool.tile([128, SL], F32, tag="r2v")
            neng.tensor_mul(r2v[:], rv[:], rv[:])
            yield
            z2t = spool.tile([128, SL], F32, tag="z2t")
            z2sc = 16.0 if o["ones_fp8"] else 1.0
            if o["ones_fp8"]:
                neng.tensor_scalar_mul(z2t[:], z2q, z2sc)
                yield
                neng.tensor_mul(z2t[:], r2v[:], z2t[:])
            else:
                neng.tensor_mul(z2t[:], r2v[:], z2q)
            yield
            Lt = wpool.tile([128, SL * K], F32, tag="Lt")
            Lt3 = Lt.rearrange("p (g c) -> p g c", c=K)
            nc.vector.tensor_tensor(Lt3[:], ntok3[:, :, 1:1 + K], bcs(rv[:]),
                                    OP.mult)
            yield
            mx = spool.tile([128, SL], F32, tag="mx")
            nc.vector.tensor_reduce(mx[:], Lt3[:], AX.X, OP.max)
            yield
            mx10 = spool.tile([128, SL], F32, tag="mx10")
            nc.vector.tensor_scalar_mul(mx10[:], mx[:], 1.0 / TEMP)
            yield
            Et = wpool.tile([128, SL * K], F32, tag="Et")
            Et3 = Et.rearrange("p (g c) -> p g c", c=K)
            weng = nc.gpsimd if o["pool_chain"] else nc.vector
            nc.vector.scalar_tensor_tensor(Et3[:], Lt3[:], 1.0 / TEMP,
                                           bcs(mx10[:]), OP.mult, OP.subtract)
            yield
            nc.scalar.activation(Et[:], Et[:], AF.Exp)
            yield
            sme = spool.tile([128, SL], F32, tag="sme")
            nc.vector.tensor_reduce(sme[:], Et3[:], AX.X, OP.add)
            yield
            rec = spool.tile([128, SL], F32, tag="rec")
            nc.vector.reciprocal(rec[:], sme[:])
            yield
            At = wpool.tile([128, SL * K], F32, tag="At")
            At3 = At.rearrange("p (g c) -> p g c", c=K)
            nc.vector.tensor_tensor(At3[:], Et3[:], bcs(rec[:]), OP.mult)
            yield
            Dt = wpool.tile([128, SL * K], F32, tag="Dt")
            Dt3 = Dt.rearrange("p (g c) -> p g c", c=K)
            nc.vector.scalar_tensor_tensor(Dt3[:], Lt3[:], -2.0, bcs(z2t[:]),
                                           OP.mult, OP.add)
            yield
            nc.vector.tensor_mul(Dt[:], Dt[:], At[:])
            yield
            At_r = At.rearrange("p (g c) -> p c g", c=K)
            Dt_r = Dt.rearrange("p (g c) -> p c g", c=K)
            if first:
                nc.vector.tensor_reduce(o_cnt[:], At_r[:], AX.X, OP.add)
                yield
                nc.vector.tensor_reduce(o_d2[:], Dt_r[:], AX.X, OP.add)
            else:
                p_cnt = spool.tile([128, K], F32, tag="p_cnt")
                nc.vector.tensor_reduce(p_cnt[:], At_r[:], AX.X, OP.add)
                yield
                nc.vector.tensor_add(o_cnt[:], o_cnt[:], p_cnt[:])
                yield
                p_d2 = spool.tile([128, K], F32, tag="p_d2")
                nc.vector.tensor_reduce(p_d2[:], Dt_r[:], AX.X, OP.add)
                yield
                nc.vector.tensor_add(o_d2[:], o_d2[:], p_d2[:])
            if finish_b is not None:
                yield
                nc.sync.dma_start(outd[finish_b, 0], o_cnt[:])
                nc.sync.dma_start(outd[finish_b, 1], o_d2[:])

        # Phase-based emission: within each super, pairs are pipelined
        # (new pair per tick, stages interleaved); token-chains are emitted
        # as separate blocks lagged one batch; input DMAs prefetch one
        # super ahead so pair-0 of super s+1 never waits on its data.
        NSUP = BPC * N // SUPER
        xt_sup = {}

        def ensure_super(si):
            if si >= NSUP or si in xt_sup:
                return
            xti = xpool.tile([128, SUPER], F16, tag="xt")
            if si == 0:
                # split the cold-start DMA so pair 0 starts ~2.4us earlier
                for q in range(4):
                    in_dma.dma_start(xti[:, q * PAIR:(q + 1) * PAIR],
                                     xt[:, q * PAIR:(q + 1) * PAIR])
            else:
                in_dma.dma_start(xti[:], xt[:, si * SUPER:(si + 1) * SUPER])
            xt_sup[si] = xti

        def emit_super(b, s, ttok3):
            si = b * (N // SUPER) + s
            ensure_super(si)
            ensure_super(si + 1)
            xt_t = xt_sup.pop(si)
            live = []
            for pr in range(SUPER // PAIR):
                gpr = s * (SUPER // PAIR) + pr
                tfeat = fpool.tile([2 * NR, MMN], F16, tag="tfeat")
                live.append(pair_stages(xt_t, pr * PAIR, tfeat,
                                        ttok3, gpr))
                nxt = []
                for g in live:
                    try:
                        next(g)
                        nxt.append(g)
                    except StopIteration:
                        pass
                live = nxt
            while live:
                nxt = []
                for g in live:
                    try:
                        next(g)
                        nxt.append(g)
                    except StopIteration:
                        pass
                live = nxt

        def new_ttok():
            ttok = tokpool.tile([128, SLOTS // 2 * TT_COLS], F16, tag="ttok")
            return ttok.rearrange("p (g c) -> p g c", c=TT_COLS)

        def emit_tokmajor(b, ttok3):
            o_cnt = opool.tile([128, K], F32, tag="o_cnt")
            o_d2 = opool.tile([128, K], F32, tag="o_d2")
            ns = o["strands"]
            step = PPB // ns
            gens = [tokmajor_strand(ttok3, i * step, (i + 1) * step,
                                    o_cnt, o_d2, i == 0,
                                    finish_b=b if i == ns - 1 else None)
                    for i in range(ns)]
            live = list(gens)
            while live:
                nxt = []
                for g in live:
                    try:
                        next(g)
                        nxt.append(g)
                    except StopIteration:
                        pass
                live = nxt

        # Schedule: per batch b, emit [super 2b][chain(b-1) part1 (through
        # Et; stops before exp)][super 2b+1][chain(b-1) part2].  Splitting
        # at exp keeps the chain's ACT ops from head-of-line blocking the
        # next batch's gelus on the in-order ACT queue.
        P1_STEPS = o["p1_steps"]

        def adv(g, steps):
            for _ in range(steps):
                try:
                    next(g)
                except StopIteration:
                    return None
            return g

        def make_chain(b, tt3, nstr):
            o_cnt = opool.tile([128, K], F32, tag="o_cnt")
            o_d2 = opool.tile([128, K], F32, tag="o_d2")
            step = PPB // nstr
            return [tokmajor_strand(tt3, i * step, (i + 1) * step,
                                    o_cnt, o_d2, i == 0,
                                    finish_b=b if i == nstr - 1 else None)
                    for i in range(nstr)]

        def drive_all(gens):
            live = [g for g in gens if g is not None]
            while live:
                nxt = []
                for g in live:
                    try:
                        next(g)
                        nxt.append(g)
                    except StopIteration:
                        pass
                live = nxt

        prev = None   # (b, [strand gens]) pending part2
        for b in range(BPC):
            tt3 = new_ttok()
            emit_super(b, 0, tt3)
            if prev is not None:
                prev = (prev[0], [adv(g, P1_STEPS) for g in prev[1]])
            emit_super(b, 1, tt3)
            if prev is not None:
                drive_all(prev[1])
            prev = (b, make_chain(b, tt3,
                                  o["strands_last"] if b == BPC - 1
                                  else o["strands"]))
        drive_all(prev[1])

    _split_excess_waits(nc)
    return nc


def kernel(x, w1, b1, ln_g, ln_b, w2, b2, prototypes):
    x = np.asarray(x, dtype=np.float32)
    w1 = np.asarray(w1, dtype=np.float32)
    b1 = np.asarray(b1, dtype=np.float32)
    ln_g = np.asarray(ln_g, dtype=np.float32)
    ln_b = np.asarray(ln_b, dtype=np.float32)
    w2 = np.asarray(w2, dtype=np.float32)
    b2 = np.asarray(b2, dtype=np.float32)
    prot = np.asarray(prototypes, dtype=np.float32)

    SU, SN, ON, cp, cc, p2 = _host_fold(w1, b1, ln_g, ln_b, w2, b2, prot)
    if max(abs(cp).max(), abs(cc)) > 1e-12:
        raise NotImplementedError(
            "nonzero ln_b/b2 path not emitted (inputs have zero bias)")

    su_np = SU.astype(np.float16)
    sn_np = SN.astype(np.float16)
    on_np = ON.astype(np.float16)
    w1_np = w1.astype(np.float16)            # [128, 64]
    b1_np = np.concatenate([b1, b1]).reshape(128, 1).astype(np.float32)

    from concourse.bass_utils import run_bass_kernel_spmd

    nc = _build_program(NCORES)
    in_maps = []
    for c in range(NCORES):
        xs = x[c * BPC:(c + 1) * BPC].reshape(T, PULSE)
        xt_np = np.ascontiguousarray(xs.T).astype(np.float16)
        in_maps.append({"xt": xt_np, "w1d": w1_np, "sud": su_np,
                        "snd": sn_np, "ond": on_np, "b1d": b1_np})

    res = run_bass_kernel_spmd(nc, in_maps, core_ids=list(range(NCORES)))

    var = np.empty((B, K), np.float32)
    for c in range(NCORES):
        o = res.results[c]["outd"].astype(np.float64)   # [BPC, 2, 128, K]
        C0 = o[:, 0].sum(axis=1)                        # [BPC, K]
        Dsum = o[:, 1].sum(axis=1)                      # [BPC, K]
        cnt = C0 + 1e-6
        v = (Dsum + cc * C0) / cnt + p2[None, :] * C0 / cnt
        var[c * BPC:(c + 1) * BPC] = v.astype(np.float32)
    return var


# revision 41
# speedup vs baseline: 1.0801x; 1.0372x over previous
"""Trainium2 Bass kernel for nn_DL_SOTA_PrototypeNet (vq_codebook).

Math restructuring (all exact, done host-side on the tiny weights):
  g   = gelu(x @ w1 + b1)                         [n, 64]
  With LN folded:  z = r * (g @ Wbar) + c  where
      Wbar = diag(ln_g) @ w2 - ones/H * (ln_g @ w2),  c = ln_b @ w2 + b2,
      r = rsqrt(var_h + eps)   (mean folds into Wbar exactly)
  logits L = r * (g @ Wp) + cp,    Wp = Wbar @ P.T, cp = c @ P.T
  |z|^2    = r^2 * sum_j (g @ E)_j^2,  E E^T = Wbar Wbar^T (eigh)
  The D=256 dimension never appears on device.

Key basis trick: with Ghat = Q diag(lam) Q^T (orthonormal Q) and
v = Q^T g:  |z_raw|^2 = sum_j lam_j v_j^2  AND  m2 = sum_j v_j^2 / 64
(Q orthonormal => |v| = |g|).  So g^2 is never computed on device; both
quadratic stats come from one squared vector via weighted-sum matmuls.

Device pipeline per core (4 batches x 8192 tokens), pair = 1024 tokens
(two 512-token chunks A/B stacked on psum partitions so every elementwise
pass runs 128 partitions wide):
  mm1: A -> hp[0:64], B -> hp[64:128]; ONE gelu [128,512]
  tail-V x2 (partition-masked Q stationaries over gp) -> V psum [128,512]
  tail-N x2 (masked [z2|Wp|mu|m2slot]) -> Np psum rows 0:32 (A), 32:64 (B)
  evac1: ONE op squares V -> qsq fp16 [128,512]
  w-mm x2 (masked [lam | 1/64] stationaries) -> z2,m2 accumulated into Np
  evac2: ONE op copies Np [64,512] -> tfeat fp16
  x-bar transpose [64,512] -> token-major [128, 4, 64] per pair
  token-major: LN scalars, softmax(L*r/T), weighted stats -> [128,K] partials
  host reduces partitions + final divide.
"""
import sys
from contextlib import ExitStack

sys.path.insert(0, "/opt/trn_rl_repo")

import numpy as np

import concourse.bass as bass
import concourse.mybir as mybir
import concourse.tile as tile
from concourse.vector_clock import ScopedClock, VectorClock

# ---------------------------------------------------------------------------
# Workaround: this walrus build only accepts 1 sync-wait per CTRL (Drain)
# instruction; Tile's tail drain carries one wait per active proc. Split it.
_orig_drain_and_barrier = tile.TileContext._drain_and_barrier


def _patched_drain_and_barrier(self, tick_clock, wait_clock):
    gclock = tick_clock.global_clock
    nprocs = len(gclock)
    procs = [i for i in range(nprocs) if gclock[i] > 0]
    for p in procs:
        vec = [gclock[i] if i == p else 0 for i in range(nprocs)]
        drain_inst = self.nc.sync.drain()
        wait_clock.add_sem_waits(drain_inst.ins, ScopedClock({None: VectorClock(vec)}))
    if not procs:
        self.nc.sync.drain()
    self.nc.all_engine_barrier()
    assert self.sems is not None
    popped = self.nc._tile_sem_poison_stack.pop()
    assert popped is self._sem_poison
    self.nc.clear_and_free_semaphores(list(self.sems.allocated().values()))
    self.nc.all_engine_barrier()


tile.TileContext._drain_and_barrier = _patched_drain_and_barrier


def _split_excess_waits(nc, max_waits=1):
    """This walrus rejects instructions with more than ~1 sync wait. Hoist
    excess waits onto same-engine NoOps placed immediately before the
    instruction (engine streams execute in order, and DMA issue happens at
    NX-execution time, so semantics are preserved)."""
    idx = 0
    for bbname, bbh in nc.bb_map.items():
        insts = bbh.bb.instructions
        out = []
        for inst in insts:
            si = getattr(inst, "sync_info", None)
            waits = list(si.on_wait) if si is not None and si.on_wait else []
            if len(waits) > max_waits:
                extra, keep = waits[:-max_waits], waits[-max_waits:]
                for w in extra:
                    nop = mybir.InstNoOp(name=f"I-waitsplit-{idx}", ins=[], outs=[])
                    idx += 1
                    nop.engine = inst.engine
                    nop.sync_info = mybir.SyncInfo(on_wait=[w], on_update=[])
                    nc.register_instruction(nop, overwrite=True)
                    out.append(nop)
                si.on_wait = keep
            out.append(inst)
        insts[:] = out
# ---------------------------------------------------------------------------

B, N, PULSE = 32, 8192, 128
H, D, K = 64, 256, 6
TEMP, LN_EPS = 0.1, 1e-5
NCORES = 8
BPC = B // NCORES              # batches per core = 4
T = BPC * N                    # tokens per core = 32768
SUPER = 4096                   # tokens per input-DMA chunk
MMN = 512                      # columns per matmul / chunk width
SLOTS = N // 128               # token slots per partition per batch = 64
PAIR = 1024                    # tokens per pair (2 x 512 chunks A/B)
PPB = N // PAIR                # pairs per batch = 8
NR = 32                        # narrow psum rows per chunk (9 used + pad;
                               # 32 so chunk B lands at matmul base 32)
TT_COLS = 2 * NR               # token-major cols per pair-slot group = 64

F16 = mybir.dt.float16
F32 = mybir.dt.float32
AF = mybir.ActivationFunctionType
OP = mybir.AluOpType
AX = mybir.AxisListType


def _host_fold(w1, b1, ln_g, ln_b, w2, b2, prot):
    f64 = np.float64
    A = ln_g.astype(f64)[:, None] * w2.astype(f64)
    a_row = ln_g.astype(f64) @ w2.astype(f64)
    c_row = ln_b.astype(f64) @ w2.astype(f64) + b2.astype(f64)
    Wbar = A - np.ones((H, 1), f64) / H * a_row[None, :]
    Wp = Wbar @ prot.T.astype(f64)            # [H, K]
    cp = c_row @ prot.T.astype(f64)           # [K]
    Ghat = Wbar @ Wbar.T
    lam, Q = np.linalg.eigh(Ghat)
    lam = np.maximum(lam, 0.0)
    cc = float(c_row @ c_row)
    p2 = np.sum(prot.astype(f64) ** 2, axis=1)  # [K]
    # tail-V stationaries [128, 2H]: cols 0:64 = Q masked to chunk-A
    # partitions (0:64), cols 64:128 = Q masked to chunk-B partitions
    SU = np.zeros((128, 2 * H), f64)
    SU[:H, 0:H] = Q
    SU[H:128, H:2 * H] = Q
    # tail-N stationaries [128, 2*NR], per chunk-mask: col 0 = z2 slot
    # (zeros; w-mm accumulates), 1:7 = Wp, 7 = mu, 8 = m2 slot (zeros)
    SN = np.zeros((128, 2 * NR), f64)
    for half in range(2):
        r0, c0 = half * H, half * NR
        SN[r0:r0 + H, c0 + 1:c0 + 1 + K] = Wp
        SN[r0:r0 + H, c0 + 7] = np.full(H, 1.0 / H)
    # w-mm stationaries [128, 2*NR]: col 0 = lam (z2), col 8 = 1/64 (m2)
    ON = np.zeros((128, 2 * NR), f64)
    for half in range(2):
        r0, c0 = half * H, half * NR
        ON[r0:r0 + H, c0 + 0] = lam
        ON[r0:r0 + H, c0 + 8] = np.full(H, 1.0 / H)
    return SU, SN, ON, cp, cc, p2


OPTS = dict(
    # evac1 (square V [128,512] -> qsq fp16) col split (act, dve)
    ev1=(512, 0),
    # evac2 (copy Np [64,512] -> tfeat) col split (act, dve)
    ev2=(64, 448),
    pool_chain=True,    # narrow token-chain muls on gpsimd
    xbufs=3, gbufs=12, fbufs=12, ttbufs=6, sbufs=5, wbufs=5, qbufs=12,
    hp_bufs=2, u_bufs=3, np_bufs=3,
    strands=1,
    strands_last=2,     # strand count for the final (exposed) chain
    p1_steps=0,         # chain steps emitted at the batch mid-boundary
    ntok_stage=True,    # stage narrow cols to f32 (gpsimd) before chain
    in_dma="sync",
    xbar_engine="sync",
    repack_eng="sync",
    ones_fp8=False,     # qsq in fp8 -> DoubleRow ones-mm (2x PE)
)


def _build_program(num_cores, opts=None):
    o = dict(OPTS)
    if opts:
        o.update(opts)
    nc = bass.Bass("TRN2", target_bir_lowering=False, debug=False,
                   num_devices=num_cores)
    # register LN_EPS so activation(bias=LN_EPS) resolves
    _eps_t = nc.alloc_sbuf_tensor(f"const-f32-eps", [128, 1], F32)
    nc.gpsimd.memset(_eps_t.ap(), LN_EPS)
    nc.const_aps.aps[(F32, LN_EPS)] = _eps_t.ap()
    nc.all_engine_barrier()
    xt = nc.dram_tensor("xt", [128, T], F16, kind="ExternalInput").ap()
    w1d = nc.dram_tensor("w1d", [128, H], F16, kind="ExternalInput").ap()
    sud = nc.dram_tensor("sud", [128, 2 * H], F16, kind="ExternalInput").ap()
    snd = nc.dram_tensor("snd", [128, 2 * NR], F16, kind="ExternalInput").ap()
    ond = nc.dram_tensor("ond", [128, 2 * NR], F16, kind="ExternalInput").ap()
    b1d = nc.dram_tensor("b1d", [128, 1], F32, kind="ExternalInput").ap()
    outd = nc.dram_tensor("outd", [BPC, 2, 128, K], F32, kind="ExternalOutput").ap()

    QT = mybir.dt.float8e4 if o["ones_fp8"] else F16

    with tile.TileContext(nc) as tc, ExitStack() as ctx:
        cpool = ctx.enter_context(tc.tile_pool(name="consts", bufs=1))
        xpool = ctx.enter_context(tc.tile_pool(name="xin", bufs=o["xbufs"]))
        hpps = ctx.enter_context(
            tc.tile_pool(name="hpps", bufs=o["hp_bufs"], space="PSUM"))
        ups = ctx.enter_context(
            tc.tile_pool(name="ups", bufs=o["u_bufs"], space="PSUM"))
        npps = ctx.enter_context(
            tc.tile_pool(name="npps", bufs=o["np_bufs"], space="PSUM"))
        gpool = ctx.enter_context(tc.tile_pool(name="gtile", bufs=o["gbufs"]))
        qpool = ctx.enter_context(tc.tile_pool(name="qsq", bufs=o["qbufs"]))
        fpool = ctx.enter_context(tc.tile_pool(name="tfeat", bufs=o["fbufs"]))
        tokpool = ctx.enter_context(tc.tile_pool(name="ttok", bufs=o["ttbufs"]))
        npool = ctx.enter_context(tc.tile_pool(name="narrow", bufs=o["ttbufs"]))
        spool = ctx.enter_context(tc.tile_pool(name="small", bufs=o["sbufs"]))
        wpool = ctx.enter_context(tc.tile_pool(name="wide", bufs=o["wbufs"]))
        opool = ctx.enter_context(tc.tile_pool(name="outs", bufs=2))

        w1sb = cpool.tile([128, H], F16, tag="w1sb")
        nc.gpsimd.dma_start(w1sb[:], w1d[:])
        susb = cpool.tile([128, 2 * H], F16, tag="susb")
        nc.gpsimd.dma_start(susb[:], sud[:])
        snsb = cpool.tile([128, 2 * NR], F16, tag="snsb")
        nc.gpsimd.dma_start(snsb[:], snd[:])
        onsb = cpool.tile([128, 2 * NR], QT, tag="onsb")
        if o["ones_fp8"]:
            onsb16 = cpool.tile([128, 2 * NR], F16, tag="onsb16")
            nc.gpsimd.dma_start(onsb16[:], ond[:])
            nc.gpsimd.tensor_copy(onsb[:], onsb16[:])
        else:
            nc.gpsimd.dma_start(onsb[:], ond[:])
        b1sb = cpool.tile([128, 1], F32, tag="b1sb")
        nc.gpsimd.dma_start(b1sb[:], b1d[:])

        xbar_eng = {"sync": nc.sync, "scalar": nc.scalar}[o["xbar_engine"]]
        in_dma = {"sync": nc.sync, "gpsimd": nc.gpsimd}[o["in_dma"]]

        def pair_stages(xt_t, xoff, tfeat, ttok3, gpr):
            """Generator: one 1024-token pair in stages; yields between
            stages so the driver can interleave pairs (in-order engine
            sequencers otherwise head-of-line block on cross-engine deps).

            Masked block-diagonal stationaries merge each chunk-A/B matmul
            pair into ONE matmul over the shared gp moving operand."""
            hp = hpps.tile([128, MMN], F32, tag="hp")
            nc.tensor.matmul(hp[0:64, :], w1sb[:],
                             xt_t[:, xoff:xoff + MMN], start=True, stop=True)
            nc.tensor.matmul(hp[64:128, :], w1sb[:],
                             xt_t[:, xoff + MMN:xoff + PAIR],
                             start=True, stop=True)
            yield
            gp = gpool.tile([128, MMN], F16, tag="gp")
            nc.scalar.activation(gp[:], hp[:], AF.Gelu, bias=b1sb[:])
            yield
            # tail-V: v_A rows 0:64, v_B rows 64:128 (one matmul)
            up = ups.tile([128, MMN], F32, tag="up")
            nc.tensor.matmul(up[:], susb[:], gp[:], start=True, stop=True)
            yield
            # evac1: square both v halves in one pass
            qsq = qpool.tile([128, MMN], QT, tag="qsq")
            a1, d1 = o["ev1"]
            if o["ones_fp8"]:
                a1, d1 = MMN, 0   # fp8 square-evac is ACT-only (scale trick)
            ev1_scale = 0.25 if o["ones_fp8"] else 1.0
            if a1:
                nc.scalar.activation(qsq[:, 0:a1], up[:, 0:a1], AF.Square,
                                     scale=ev1_scale)
            if d1:
                # DVE cannot dual-read one PSUM AP: copy, square in place
                nc.vector.tensor_copy(qsq[:, a1:MMN], up[:, a1:MMN])
                nc.vector.tensor_mul(qsq[:, a1:MMN], qsq[:, a1:MMN],
                                     qsq[:, a1:MMN])
            yield
            # tail-N (one matmul) then w-mm accumulating z2/m2 (one matmul)
            npt = npps.tile([64, MMN], F32, tag="npt")
            nc.tensor.matmul(npt[:], snsb[:], gp[:], start=True, stop=False)
            nc.tensor.matmul(npt[:], onsb[:], qsq[:], start=False, stop=True)
            yield
            # evac2: both chunks' narrow rows in ONE pass [64, 512]
            a2, d2 = o["ev2"]
            if a2:
                nc.scalar.copy(tfeat[:, 0:a2], npt[:, 0:a2])
            if d2:
                nc.vector.tensor_copy(tfeat[:, a2:MMN], npt[:, a2:MMN])
            yield
            xbar_eng.dma_start_transpose(
                ttok3[:, gpr * 4:(gpr + 1) * 4, :], tfeat[:])

        def _adv(gens, steps=1):
            nxt = []
            for g, w in gens:
                alive = True
                for _ in range(steps if w else 1):
                    try:
                        next(g)
                    except StopIteration:
                        alive = False
                        break
                if alive:
                    nxt.append((g, w))
            gens[:] = nxt

        def tokmajor_strand(ttok3, pr0, pr1, o_cnt, o_d2, first,
                            finish_b=None):
            """Generator emitting one pair-range's token-major chain.
            ttok3: [128, 32, 32] (pair-slot groups of 4, cols (i, r)).
            Token (pr, i, s, p) value r at ttok3[p, 4*pr + s, 16*i + r]."""
            NPR = pr1 - pr0
            SL = NPR * 8  # logical slots (pr, s, i)
            tt = ttok3[:, pr0 * 4:pr1 * 4, :]

            def bcs(ap_2d):
                return ap_2d.rearrange("p (g c) -> p g c", c=1).to_broadcast(
                    (128, SL, K))

            NN = 9  # narrow cols: z2raw | 6 L' | mu | m2
            # slot' = (pr, s, i): c = 32*i + r, uniform stride merge
            tt4 = tt.rearrange("p g (i r) -> p (g i) r", i=2)
            if o["ntok_stage"]:
                ntok = npool.tile([128, SL * NN], F32, tag="ntok")
                ntok3 = ntok.rearrange("p (g c) -> p g c", c=NN)
                nc.gpsimd.tensor_copy(ntok3[:], tt4[:, :, 0:NN])
                yield
            else:
                ntok3 = tt4[:, :, 0:NN]
            z2q = ntok3[:, :, 0]
            muv = ntok3[:, :, 7]
            m2v = ntok3[:, :, 8]
            neng = nc.gpsimd if o["pool_chain"] else nc.vector
            vvar = spool.tile([128, SL], F32, tag="vvar")
            neng.tensor_mul(vvar[:], muv, muv)   # mu^2
            yield
            neng.tensor_sub(vvar[:], m2v, vvar[:])
            yield
            sqv = spool.tile([128, SL], F32, tag="sqv")
            nc.scalar.activation(sqv[:], vvar[:], AF.Sqrt, bias=LN_EPS)
            yield
            rv = spool.tile([128, SL], F32, tag="rv")
            nc.vector.reciprocal(rv[:], sqv[:])
            yield
            r2v = spool.tile([128, SL], F32, tag="r2v")
            neng.tensor_mul(r2v[:], rv[:], rv[:])
            yield
            z2t = spool.tile([128, SL], F32, tag="z2t")
            z2sc = 16.0 if o["ones_fp8"] else 1.0
            if o["ones_fp8"]:
                neng.tensor_scalar_mul(z2t[:], z2q, z2sc)
                yield
                neng.tensor_mul(z2t[:], r2v[:], z2t[:])
            else:
                neng.tensor_mul(z2t[:], r2v[:], z2q)
            yield
            Lt = wpool.tile([128, SL * K], F32, tag="Lt")
            Lt3 = Lt.rearrange("p (g c) -> p g c", c=K)
            nc.vector.tensor_tensor(Lt3[:], ntok3[:, :, 1:1 + K], bcs(rv[:]),
                                    OP.mult)
            yield
            mx = spool.tile([128, SL], F32, tag="mx")
            nc.vector.tensor_reduce(mx[:], Lt3[:], AX.X, OP.max)
            yield
            mx10 = spool.tile([128, SL], F32, tag="mx10")
            nc.vector.tensor_scalar_mul(mx10[:], mx[:], 1.0 / TEMP)
            yield
            Et = wpool.tile([128, SL * K], F32, tag="Et")
            Et3 = Et.rearrange("p (g c) -> p g c", c=K)
            weng = nc.gpsimd if o["pool_chain"] else nc.vector
            nc.vector.scalar_tensor_tensor(Et3[:], Lt3[:], 1.0 / TEMP,
                                           bcs(mx10[:]), OP.mult, OP.subtract)
            yield
            nc.scalar.activation(Et[:], Et[:], AF.Exp)
            yield
            sme = spool.tile([128, SL], F32, tag="sme")
            nc.vector.tensor_reduce(sme[:], Et3[:], AX.X, OP.add)
            yield
            rec = spool.tile([128, SL], F32, tag="rec")
            nc.vector.reciprocal(rec[:], sme[:])
            yield
            At = wpool.tile([128, SL * K], F32, tag="At")
            At3 = At.rearrange("p (g c) -> p g c", c=K)
            nc.vector.tensor_tensor(At3[:], Et3[:], bcs(rec[:]), OP.mult)
            yield
            Dt = wpool.tile([128, SL * K], F32, tag="Dt")
            Dt3 = Dt.rearrange("p (g c) -> p g c", c=K)
            nc.vector.scalar_tensor_tensor(Dt3[:], Lt3[:], -2.0, bcs(z2t[:]),
                                           OP.mult, OP.add)
            yield
            nc.vector.tensor_mul(Dt[:], Dt[:], At[:])
            yield
            At_r = At.rearrange("p (g c) -> p c g", c=K)
            Dt_r = Dt.rearrange("p (g c) -> p c g", c=K)
            if first:
                nc.vector.tensor_reduce(o_cnt[:], At_r[:], AX.X, OP.add)
                yield
                nc.vector.tensor_reduce(o_d2[:], Dt_r[:], AX.X, OP.add)
            else:
                p_cnt = spool.tile([128, K], F32, tag="p_cnt")
                nc.vector.tensor_reduce(p_cnt[:], At_r[:], AX.X, OP.add)
                yield
                nc.vector.tensor_add(o_cnt[:], o_cnt[:], p_cnt[:])
                yield
                p_d2 = spool.tile([128, K], F32, tag="p_d2")
                nc.vector.tensor_reduce(p_d2[:], Dt_r[:], AX.X, OP.add)
                yield
                nc.vector.tensor_add(o_d2[:], o_d2[:], p_d2[:])
            if finish_b is not None:
                yield
                nc.sync.dma_start(outd[finish_b, 0], o_cnt[:])
                nc.sync.dma_start(outd[finish_b, 1], o_d2[:])

        # Phase-based emission: within each super, pairs are pipelined
        # (new pair per tick, stages interleaved); token-chains are emitted
        # as separate blocks lagged one batch; input DMAs prefetch one
        # super ahead so pair-0 of super s+1 never waits on its data.
        NSUP = BPC * N // SUPER
        xt_sup = {}

        def ensure_super(si):
            if si >= NSUP or si in xt_sup:
                return
            xti = xpool.tile([128, SUPER], F16, tag="xt")
            if si == 0:
                # split the cold-start DMA so pair 0 starts ~2.4us earlier
                for q in range(4):
                    in_dma.dma_start(xti[:, q * PAIR:(q + 1) * PAIR],
                                     xt[:, q * PAIR:(q + 1) * PAIR])
            else:
                in_dma.dma_start(xti[:], xt[:, si * SUPER:(si + 1) * SUPER])
            xt_sup[si] = xti

        def emit_super(b, s, ttok3):
            si = b * (N // SUPER) + s
            ensure_super(si)
            ensure_super(si + 1)
            xt_t = xt_sup.pop(si)
            live = []
            for pr in range(SUPER // PAIR):
                gpr = s * (SUPER // PAIR) + pr
                tfeat = fpool.tile([2 * NR, MMN], F16, tag="tfeat")
                live.append(pair_stages(xt_t, pr * PAIR, tfeat,
                                        ttok3, gpr))
                nxt = []
                for g in live:
                    try:
                        next(g)
                        nxt.append(g)
                    except StopIteration:
                        pass
                live = nxt
            while live:
                nxt = []
                for g in live:
                    try:
                        next(g)
                        nxt.append(g)
                    except StopIteration:
                        pass
                live = nxt

        def new_ttok():
            ttok = tokpool.tile([128, SLOTS // 2 * TT_COLS], F16, tag="ttok")
            return ttok.rearrange("p (g c) -> p g c", c=TT_COLS)

        def emit_tokmajor(b, ttok3):
            o_cnt = opool.tile([128, K], F32, tag="o_cnt")
            o_d2 = opool.tile([128, K], F32, tag="o_d2")
            ns = o["strands"]
            step = PPB // ns
            gens = [tokmajor_strand(ttok3, i * step, (i + 1) * step,
                                    o_cnt, o_d2, i == 0,
                                    finish_b=b if i == ns - 1 else None)
                    for i in range(ns)]
            live = list(gens)
            while live:
                nxt = []
                for g in live:
                    try:
                        next(g)
                        nxt.append(g)
                    except StopIteration:
                        pass
                live = nxt

        # Schedule: per batch b, emit [super 2b][chain(b-1) part1 (through
        # Et; stops before exp)][super 2b+1][chain(b-1) part2].  Splitting
        # at exp keeps the chain's ACT ops from head-of-line blocking the
        # next batch's gelus on the in-order ACT queue.
        P1_STEPS = o["p1_steps"]

        def adv(g, steps):
            for _ in range(steps):
                try:
                    next(g)
                except StopIteration:
                    return None
            return g

        def make_chain(b, tt3, nstr):
            o_cnt = opool.tile([128, K], F32, tag="o_cnt")
            o_d2 = opool.tile([128, K], F32, tag="o_d2")
            step = PPB // nstr
            return [tokmajor_strand(tt3, i * step, (i + 1) * step,
                                    o_cnt, o_d2, i == 0,
                                    finish_b=b if i == nstr - 1 else None)
                    for i in range(nstr)]

        def drive_all(gens):
            live = [g for g in gens if g is not None]
            while live:
                nxt = []
                for g in live:
                    try:
                        next(g)
                        nxt.append(g)
                    except StopIteration:
                        pass
                live = nxt

        prev = None   # (b, [strand gens]) pending part2
        for b in range(BPC):
            tt3 = new_ttok()
            emit_super(b, 0, tt3)
            if prev is not None:
                prev = (prev[0], [adv(g, P1_STEPS) for g in prev[1]])
            emit_super(b, 1, tt3)
            if prev is not None:
                drive_all(prev[1])
            prev = (b, make_chain(b, tt3,
                                  o["strands_last"] if b == BPC - 1
                                  else o["strands"]))
        drive_all(prev[1])

    _split_excess_waits(nc)
    return nc


def kernel(x, w1, b1, ln_g, ln_b, w2, b2, prototypes):
    x = np.asarray(x, dtype=np.float32)
    w1 = np.asarray(w1, dtype=np.float32)
    b1 = np.asarray(b1, dtype=np.float32)
    ln_g = np.asarray(ln_g, dtype=np.float32)
    ln_b = np.asarray(ln_b, dtype=np.float32)
    w2 = np.asarray(w2, dtype=np.float32)
    b2 = np.asarray(b2, dtype=np.float32)
    prot = np.asarray(prototypes, dtype=np.float32)

    SU, SN, ON, cp, cc, p2 = _host_fold(w1, b1, ln_g, ln_b, w2, b2, prot)
    if max(abs(cp).max(), abs(cc)) > 1e-12:
        raise NotImplementedError(
            "nonzero ln_b/b2 path not emitted (inputs have zero bias)")

    su_np = SU.astype(np.float16)
    sn_np = SN.astype(np.float16)
    on_np = ON.astype(np.float16)
    w1_np = w1.astype(np.float16)            # [128, 64]
    b1_np = np.concatenate([b1, b1]).reshape(128, 1).astype(np.float32)

    from concourse.bass_utils import run_bass_kernel_spmd

    nc = _build_program(NCORES)
    in_maps = []
    for c in range(NCORES):
        xs = x[c * BPC:(c + 1) * BPC].reshape(T, PULSE)
        xt_np = np.ascontiguousarray(xs.T).astype(np.float16)
        in_maps.append({"xt": xt_np, "w1d": w1_np, "sud": su_np,
                        "snd": sn_np, "ond": on_np, "b1d": b1_np})

    res = run_bass_kernel_spmd(nc, in_maps, core_ids=list(range(NCORES)))

    var = np.empty((B, K), np.float32)
    for c in range(NCORES):
        o = res.results[c]["outd"].astype(np.float64)   # [BPC, 2, 128, K]
        C0 = o[:, 0].sum(axis=1)                        # [BPC, K]
        Dsum = o[:, 1].sum(axis=1)                      # [BPC, K]
        cnt = C0 + 1e-6
        v = (Dsum + cc * C0) / cnt + p2[None, :] * C0 / cnt
        var[c * BPC:(c + 1) * BPC] = v.astype(np.float32)
    return var


# revision 42
# speedup vs baseline: 1.1642x; 1.0779x over previous
"""Trainium2 Bass kernel for nn_DL_SOTA_PrototypeNet (vq_codebook).

Math restructuring (all exact, done host-side on the tiny weights):
  g   = gelu(x @ w1 + b1)                         [n, 64]
  With LN folded:  z = r * (g @ Wbar) + c  where
      Wbar = diag(ln_g) @ w2 - ones/H * (ln_g @ w2),  c = ln_b @ w2 + b2,
      r = rsqrt(var_h + eps)   (mean folds into Wbar exactly)
  logits L = r * (g @ Wp) + cp,    Wp = Wbar @ P.T, cp = c @ P.T
  |z|^2    = r^2 * sum_j (g @ E)_j^2,  E E^T = Wbar Wbar^T (eigh)
  The D=256 dimension never appears on device.

Key basis trick: with Ghat = Q diag(lam) Q^T (orthonormal Q) and
v = Q^T g:  |z_raw|^2 = sum_j lam_j v_j^2  AND  m2 = sum_j v_j^2 / 64
(Q orthonormal => |v| = |g|).  So g^2 is never computed on device; both
quadratic stats come from one squared vector via weighted-sum matmuls.

Device pipeline per core (4 batches x 8192 tokens), pair = 1024 tokens
(two 512-token chunks A/B stacked on psum partitions so every elementwise
pass runs 128 partitions wide):
  mm1: A -> hp[0:64], B -> hp[64:128]; ONE gelu [128,512]
  tail-V x2 (partition-masked Q stationaries over gp) -> V psum [128,512]
  tail-N x2 (masked [z2|Wp|mu|m2slot]) -> Np psum rows 0:32 (A), 32:64 (B)
  evac1: ONE op squares V -> qsq fp16 [128,512]
  w-mm x2 (masked [lam | 1/64] stationaries) -> z2,m2 accumulated into Np
  evac2: ONE op copies Np [64,512] -> tfeat fp16
  x-bar transpose [64,512] -> token-major [128, 4, 64] per pair
  token-major: LN scalars, softmax(L*r/T), weighted stats -> [128,K] partials
  host reduces partitions + final divide.
"""
import sys
from contextlib import ExitStack

sys.path.insert(0, "/opt/trn_rl_repo")

import numpy as np

import concourse.bass as bass
import concourse.mybir as mybir
import concourse.tile as tile
from concourse.vector_clock import ScopedClock, VectorClock

# ---------------------------------------------------------------------------
# Workaround: this walrus build only accepts 1 sync-wait per CTRL (Drain)
# instruction; Tile's tail drain carries one wait per active proc. Split it.
_orig_drain_and_barrier = tile.TileContext._drain_and_barrier


def _patched_drain_and_barrier(self, tick_clock, wait_clock):
    gclock = tick_clock.global_clock
    nprocs = len(gclock)
    procs = [i for i in range(nprocs) if gclock[i] > 0]
    for p in procs:
        vec = [gclock[i] if i == p else 0 for i in range(nprocs)]
        drain_inst = self.nc.sync.drain()
        wait_clock.add_sem_waits(drain_inst.ins, ScopedClock({None: VectorClock(vec)}))
    if not procs:
        self.nc.sync.drain()
    self.nc.all_engine_barrier()
    assert self.sems is not None
    popped = self.nc._tile_sem_poison_stack.pop()
    assert popped is self._sem_poison
    self.nc.clear_and_free_semaphores(list(self.sems.allocated().values()))
    self.nc.all_engine_barrier()


tile.TileContext._drain_and_barrier = _patched_drain_and_barrier


def _split_excess_waits(nc, max_waits=1):
    """This walrus rejects instructions with more than ~1 sync wait. Hoist
    excess waits onto same-engine NoOps placed immediately before the
    instruction (engine streams execute in order, and DMA issue happens at
    NX-execution time, so semantics are preserved)."""
    idx = 0
    for bbname, bbh in nc.bb_map.items():
        insts = bbh.bb.instructions
        out = []
        for inst in insts:
            si = getattr(inst, "sync_info", None)
            waits = list(si.on_wait) if si is not None and si.on_wait else []
            if len(waits) > max_waits:
                extra, keep = waits[:-max_waits], waits[-max_waits:]
                for w in extra:
                    nop = mybir.InstNoOp(name=f"I-waitsplit-{idx}", ins=[], outs=[])
                    idx += 1
                    nop.engine = inst.engine
                    nop.sync_info = mybir.SyncInfo(on_wait=[w], on_update=[])
                    nc.register_instruction(nop, overwrite=True)
                    out.append(nop)
                si.on_wait = keep
            out.append(inst)
        insts[:] = out
# ---------------------------------------------------------------------------

B, N, PULSE = 32, 8192, 128
H, D, K = 64, 256, 6
TEMP, LN_EPS = 0.1, 1e-5
NCORES = 8
BPC = B // NCORES              # batches per core = 4
T = BPC * N                    # tokens per core = 32768
SUPER = 4096                   # tokens per input-DMA chunk
MMN = 512                      # columns per matmul / chunk width
SLOTS = N // 128               # token slots per partition per batch = 64
PAIR = 1024                    # tokens per pair (2 x 512 chunks A/B)
PPB = N // PAIR                # pairs per batch = 8
NR = 32                        # narrow psum rows per chunk (9 used + pad;
                               # 32 so chunk B lands at matmul base 32)
TT_COLS = 2 * NR               # token-major cols per pair-slot group = 64

F16 = mybir.dt.float16
F32 = mybir.dt.float32
AF = mybir.ActivationFunctionType
OP = mybir.AluOpType
AX = mybir.AxisListType


def _host_fold(w1, b1, ln_g, ln_b, w2, b2, prot):
    f64 = np.float64
    A = ln_g.astype(f64)[:, None] * w2.astype(f64)
    a_row = ln_g.astype(f64) @ w2.astype(f64)
    c_row = ln_b.astype(f64) @ w2.astype(f64) + b2.astype(f64)
    Wbar = A - np.ones((H, 1), f64) / H * a_row[None, :]
    Wp = Wbar @ prot.T.astype(f64)            # [H, K]
    cp = c_row @ prot.T.astype(f64)           # [K]
    Ghat = Wbar @ Wbar.T
    lam, Q = np.linalg.eigh(Ghat)
    lam = np.maximum(lam, 0.0)
    cc = float(c_row @ c_row)
    p2 = np.sum(prot.astype(f64) ** 2, axis=1)  # [K]
    # tail-V stationaries [128, 2H]: cols 0:64 = Q masked to chunk-A
    # partitions (0:64), cols 64:128 = Q masked to chunk-B partitions
    SU = np.zeros((128, 2 * H), f64)
    SU[:H, 0:H] = Q
    SU[H:128, H:2 * H] = Q
    # tail-N stationaries [128, 2*NR], per chunk-mask: col 0 = z2 slot
    # (zeros; w-mm accumulates), 1:7 = Wp, 7 = mu, 8 = m2 slot (zeros)
    SN = np.zeros((128, 2 * NR), f64)
    for half in range(2):
        r0, c0 = half * H, half * NR
        SN[r0:r0 + H, c0 + 1:c0 + 1 + K] = Wp
        SN[r0:r0 + H, c0 + 7] = np.full(H, 1.0 / H)
    # w-mm stationaries [128, 2*NR]: col 0 = lam (z2), col 8 = 1/64 (m2)
    ON = np.zeros((128, 2 * NR), f64)
    for half in range(2):
        r0, c0 = half * H, half * NR
        ON[r0:r0 + H, c0 + 0] = lam
        ON[r0:r0 + H, c0 + 8] = np.full(H, 1.0 / H)
    return SU, SN, ON, cp, cc, p2


OPTS = dict(
    # evac1 (square V [128,512] -> qsq fp16) col split (act, dve)
    ev1=(512, 0),
    # evac2 (copy Np [64,512] -> tfeat) col split (act, dve)
    ev2=(64, 448),
    pool_chain=True,    # narrow token-chain muls on gpsimd
    xbufs=3, gbufs=12, fbufs=12, ttbufs=6, sbufs=5, wbufs=5, qbufs=12,
    hp_bufs=2, u_bufs=3, np_bufs=3,
    strands=1,
    strands_last=2,     # strand count for the final (exposed) chain
    p1_steps=0,         # chain steps emitted at the batch mid-boundary
    ntok_stage=True,    # stage narrow cols to f32 (gpsimd) before chain
    in_dma="sync",
    xbar_engine="sync",
    repack_eng="sync",
    ones_fp8=False,     # qsq in fp8 -> DoubleRow ones-mm (2x PE)
)


def _build_program(num_cores, opts=None):
    o = dict(OPTS)
    if opts:
        o.update(opts)
    nc = bass.Bass("TRN2", target_bir_lowering=False, debug=False,
                   num_devices=num_cores)
    # register LN_EPS so activation(bias=LN_EPS) resolves
    _eps_t = nc.alloc_sbuf_tensor(f"const-f32-eps", [128, 1], F32)
    nc.gpsimd.memset(_eps_t.ap(), LN_EPS)
    nc.const_aps.aps[(F32, LN_EPS)] = _eps_t.ap()
    nc.all_engine_barrier()
    xt = nc.dram_tensor("xt", [128, T], F16, kind="ExternalInput").ap()
    w1d = nc.dram_tensor("w1d", [128, H], F16, kind="ExternalInput").ap()
    sud = nc.dram_tensor("sud", [128, 2 * H], F16, kind="ExternalInput").ap()
    snd = nc.dram_tensor("snd", [128, 2 * NR], F16, kind="ExternalInput").ap()
    ond = nc.dram_tensor("ond", [128, 2 * NR], F16, kind="ExternalInput").ap()
    b1d = nc.dram_tensor("b1d", [128, 1], F32, kind="ExternalInput").ap()
    outd = nc.dram_tensor("outd", [BPC, 2, 128, K], F32, kind="ExternalOutput").ap()

    QT = mybir.dt.float8e4 if o["ones_fp8"] else F16

    with tile.TileContext(nc) as tc, ExitStack() as ctx:
        cpool = ctx.enter_context(tc.tile_pool(name="consts", bufs=1))
        xpool = ctx.enter_context(tc.tile_pool(name="xin", bufs=o["xbufs"]))
        hpps = ctx.enter_context(
            tc.tile_pool(name="hpps", bufs=o["hp_bufs"], space="PSUM"))
        ups = ctx.enter_context(
            tc.tile_pool(name="ups", bufs=o["u_bufs"], space="PSUM"))
        npps = ctx.enter_context(
            tc.tile_pool(name="npps", bufs=o["np_bufs"], space="PSUM"))
        gpool = ctx.enter_context(tc.tile_pool(name="gtile", bufs=o["gbufs"]))
        qpool = ctx.enter_context(tc.tile_pool(name="qsq", bufs=o["qbufs"]))
        fpool = ctx.enter_context(tc.tile_pool(name="tfeat", bufs=o["fbufs"]))
        tokpool = ctx.enter_context(tc.tile_pool(name="ttok", bufs=o["ttbufs"]))
        npool = ctx.enter_context(tc.tile_pool(name="narrow", bufs=o["ttbufs"]))
        spool = ctx.enter_context(tc.tile_pool(name="small", bufs=o["sbufs"]))
        wpool = ctx.enter_context(tc.tile_pool(name="wide", bufs=o["wbufs"]))
        opool = ctx.enter_context(tc.tile_pool(name="outs", bufs=2))

        w1sb = cpool.tile([128, H], F16, tag="w1sb")
        nc.gpsimd.dma_start(w1sb[:], w1d[:])
        susb = cpool.tile([128, 2 * H], F16, tag="susb")
        nc.gpsimd.dma_start(susb[:], sud[:])
        snsb = cpool.tile([128, 2 * NR], F16, tag="snsb")
        nc.gpsimd.dma_start(snsb[:], snd[:])
        onsb = cpool.tile([128, 2 * NR], QT, tag="onsb")
        if o["ones_fp8"]:
            onsb16 = cpool.tile([128, 2 * NR], F16, tag="onsb16")
            nc.gpsimd.dma_start(onsb16[:], ond[:])
            nc.gpsimd.tensor_copy(onsb[:], onsb16[:])
        else:
            nc.gpsimd.dma_start(onsb[:], ond[:])
        b1sb = cpool.tile([128, 1], F32, tag="b1sb")
        nc.gpsimd.dma_start(b1sb[:], b1d[:])

        xbar_eng = {"sync": nc.sync, "scalar": nc.scalar}[o["xbar_engine"]]
        in_dma = {"sync": nc.sync, "gpsimd": nc.gpsimd}[o["in_dma"]]

        def pair_stages(xt_t, xoff, tfeat, col0):
            """Generator: one 1024-token pair in stages; yields between
            stages so the driver can interleave pairs (in-order engine
            sequencers otherwise head-of-line block on cross-engine deps).

            Masked block-diagonal stationaries merge each chunk-A/B matmul
            pair into ONE matmul over the shared gp moving operand."""
            hp = hpps.tile([128, MMN], F32, tag="hp")
            nc.tensor.matmul(hp[0:64, :], w1sb[:],
                             xt_t[:, xoff:xoff + MMN], start=True, stop=True)
            nc.tensor.matmul(hp[64:128, :], w1sb[:],
                             xt_t[:, xoff + MMN:xoff + PAIR],
                             start=True, stop=True)
            yield
            gp = gpool.tile([128, MMN], F16, tag="gp")
            nc.scalar.activation(gp[:], hp[:], AF.Gelu, bias=b1sb[:])
            yield
            # tail-V: v_A rows 0:64, v_B rows 64:128 (one matmul)
            up = ups.tile([128, MMN], F32, tag="up")
            nc.tensor.matmul(up[:], susb[:], gp[:], start=True, stop=True)
            yield
            # evac1: square both v halves in one pass
            qsq = qpool.tile([128, MMN], QT, tag="qsq")
            a1, d1 = o["ev1"]
            if o["ones_fp8"]:
                a1, d1 = MMN, 0   # fp8 square-evac is ACT-only (scale trick)
            ev1_scale = 0.25 if o["ones_fp8"] else 1.0
            if a1:
                nc.scalar.activation(qsq[:, 0:a1], up[:, 0:a1], AF.Square,
                                     scale=ev1_scale)
            if d1:
                # DVE cannot dual-read one PSUM AP: copy, square in place
                nc.vector.tensor_copy(qsq[:, a1:MMN], up[:, a1:MMN])
                nc.vector.tensor_mul(qsq[:, a1:MMN], qsq[:, a1:MMN],
                                     qsq[:, a1:MMN])
            yield
            # tail-N (one matmul) then w-mm accumulating z2/m2 (one matmul)
            npt = npps.tile([64, MMN], F32, tag="npt")
            nc.tensor.matmul(npt[:], snsb[:], gp[:], start=True, stop=False)
            nc.tensor.matmul(npt[:], onsb[:], qsq[:], start=False, stop=True)
            yield
            # evac2: both chunks' narrow rows in ONE pass [64, 512]
            a2, d2 = o["ev2"]
            if a2:
                nc.scalar.copy(tfeat[:, col0:col0 + a2], npt[:, 0:a2])
            if d2:
                nc.vector.tensor_copy(tfeat[:, col0 + a2:col0 + MMN],
                                      npt[:, a2:MMN])

        def _adv(gens, steps=1):
            nxt = []
            for g, w in gens:
                alive = True
                for _ in range(steps if w else 1):
                    try:
                        next(g)
                    except StopIteration:
                        alive = False
                        break
                if alive:
                    nxt.append((g, w))
            gens[:] = nxt

        def tokmajor_strand(ttok3, pr0, pr1, o_cnt, o_d2, first,
                            finish_b=None):
            """Generator emitting one pair-range's token-major chain.
            ttok3: [128, 32, 32] (pair-slot groups of 4, cols (i, r)).
            Token (pr, i, s, p) value r at ttok3[p, 4*pr + s, 16*i + r]."""
            NPR = pr1 - pr0
            SL = NPR * 8  # logical slots (pr, s, i)
            tt = ttok3[:, pr0 * 4:pr1 * 4, :]

            def bcs(ap_2d):
                return ap_2d.rearrange("p (g c) -> p g c", c=1).to_broadcast(
                    (128, SL, K))

            NN = 9  # narrow cols: z2raw | 6 L' | mu | m2
            # slot' = (pr, s, i): c = 32*i + r, uniform stride merge
            tt4 = tt.rearrange("p g (i r) -> p (g i) r", i=2)
            if o["ntok_stage"]:
                ntok = npool.tile([128, SL * NN], F32, tag="ntok")
                ntok3 = ntok.rearrange("p (g c) -> p g c", c=NN)
                nc.gpsimd.tensor_copy(ntok3[:], tt4[:, :, 0:NN])
                yield
            else:
                ntok3 = tt4[:, :, 0:NN]
            z2q = ntok3[:, :, 0]
            muv = ntok3[:, :, 7]
            m2v = ntok3[:, :, 8]
            neng = nc.gpsimd if o["pool_chain"] else nc.vector
            vvar = spool.tile([128, SL], F32, tag="vvar")
            neng.tensor_mul(vvar[:], muv, muv)   # mu^2
            yield
            neng.tensor_sub(vvar[:], m2v, vvar[:])
            yield
            sqv = spool.tile([128, SL], F32, tag="sqv")
            nc.scalar.activation(sqv[:], vvar[:], AF.Sqrt, bias=LN_EPS)
            yield
            rv = spool.tile([128, SL], F32, tag="rv")
            nc.vector.reciprocal(rv[:], sqv[:])
            yield
            r2v = spool.tile([128, SL], F32, tag="r2v")
            neng.tensor_mul(r2v[:], rv[:], rv[:])
            yield
            z2t = spool.tile([128, SL], F32, tag="z2t")
            z2sc = 16.0 if o["ones_fp8"] else 1.0
            if o["ones_fp8"]:
                neng.tensor_scalar_mul(z2t[:], z2q, z2sc)
                yield
                neng.tensor_mul(z2t[:], r2v[:], z2t[:])
            else:
                neng.tensor_mul(z2t[:], r2v[:], z2q)
            yield
            Lt = wpool.tile([128, SL * K], F32, tag="Lt")
            Lt3 = Lt.rearrange("p (g c) -> p g c", c=K)
            nc.vector.tensor_tensor(Lt3[:], ntok3[:, :, 1:1 + K], bcs(rv[:]),
                                    OP.mult)
            yield
            mx = spool.tile([128, SL], F32, tag="mx")
            nc.vector.tensor_reduce(mx[:], Lt3[:], AX.X, OP.max)
            yield
            mx10 = spool.tile([128, SL], F32, tag="mx10")
            nc.vector.tensor_scalar_mul(mx10[:], mx[:], 1.0 / TEMP)
            yield
            Et = wpool.tile([128, SL * K], F32, tag="Et")
            Et3 = Et.rearrange("p (g c) -> p g c", c=K)
            weng = nc.gpsimd if o["pool_chain"] else nc.vector
            nc.vector.scalar_tensor_tensor(Et3[:], Lt3[:], 1.0 / TEMP,
                                           bcs(mx10[:]), OP.mult, OP.subtract)
            yield
            nc.scalar.activation(Et[:], Et[:], AF.Exp)
            yield
            sme = spool.tile([128, SL], F32, tag="sme")
            nc.vector.tensor_reduce(sme[:], Et3[:], AX.X, OP.add)
            yield
            rec = spool.tile([128, SL], F32, tag="rec")
            nc.vector.reciprocal(rec[:], sme[:])
            yield
            At = wpool.tile([128, SL * K], F32, tag="At")
            At3 = At.rearrange("p (g c) -> p g c", c=K)
            nc.vector.tensor_tensor(At3[:], Et3[:], bcs(rec[:]), OP.mult)
            yield
            Dt = wpool.tile([128, SL * K], F32, tag="Dt")
            Dt3 = Dt.rearrange("p (g c) -> p g c", c=K)
            nc.vector.scalar_tensor_tensor(Dt3[:], Lt3[:], -2.0, bcs(z2t[:]),
                                           OP.mult, OP.add)
            yield
            nc.vector.tensor_mul(Dt[:], Dt[:], At[:])
            yield
            At_r = At.rearrange("p (g c) -> p c g", c=K)
            Dt_r = Dt.rearrange("p (g c) -> p c g", c=K)
            if first:
                nc.vector.tensor_reduce(o_cnt[:], At_r[:], AX.X, OP.add)
                yield
                nc.vector.tensor_reduce(o_d2[:], Dt_r[:], AX.X, OP.add)
            else:
                p_cnt = spool.tile([128, K], F32, tag="p_cnt")
                nc.vector.tensor_reduce(p_cnt[:], At_r[:], AX.X, OP.add)
                yield
                nc.vector.tensor_add(o_cnt[:], o_cnt[:], p_cnt[:])
                yield
                p_d2 = spool.tile([128, K], F32, tag="p_d2")
                nc.vector.tensor_reduce(p_d2[:], Dt_r[:], AX.X, OP.add)
                yield
                nc.vector.tensor_add(o_d2[:], o_d2[:], p_d2[:])
            if finish_b is not None:
                yield
                nc.sync.dma_start(outd[finish_b, 0], o_cnt[:])
                nc.sync.dma_start(outd[finish_b, 1], o_d2[:])

        # Phase-based emission: within each super, pairs are pipelined
        # (new pair per tick, stages interleaved); token-chains are emitted
        # as separate blocks lagged one batch; input DMAs prefetch one
        # super ahead so pair-0 of super s+1 never waits on its data.
        NSUP = BPC * N // SUPER
        xt_sup = {}

        def ensure_super(si):
            if si >= NSUP or si in xt_sup:
                return
            xti = xpool.tile([128, SUPER], F16, tag="xt")
            if si == 0:
                # split the cold-start DMA so pair 0 starts ~2.4us earlier
                for q in range(4):
                    in_dma.dma_start(xti[:, q * PAIR:(q + 1) * PAIR],
                                     xt[:, q * PAIR:(q + 1) * PAIR])
            else:
                in_dma.dma_start(xti[:], xt[:, si * SUPER:(si + 1) * SUPER])
            xt_sup[si] = xti

        PPS = SUPER // PAIR   # pairs per super = 4

        def emit_super(b, s, ttok3):
            si = b * (N // SUPER) + s
            ensure_super(si)
            ensure_super(si + 1)
            xt_t = xt_sup.pop(si)
            tfeat = fpool.tile([2 * NR, PPS * MMN], F16, tag="tfeat")
            live = []
            for pr in range(PPS):
                live.append(pair_stages(xt_t, pr * PAIR, tfeat, pr * MMN))
                nxt = []
                for g in live:
                    try:
                        next(g)
                        nxt.append(g)
                    except StopIteration:
                        pass
                live = nxt
            while live:
                nxt = []
                for g in live:
                    try:
                        next(g)
                        nxt.append(g)
                    except StopIteration:
                        pass
                live = nxt
            xbar_eng.dma_start_transpose(
                ttok3[:, s * 4 * PPS:(s + 1) * 4 * PPS, :], tfeat[:])

        def new_ttok():
            ttok = tokpool.tile([128, SLOTS // 2 * TT_COLS], F16, tag="ttok")
            return ttok.rearrange("p (g c) -> p g c", c=TT_COLS)

        def emit_tokmajor(b, ttok3):
            o_cnt = opool.tile([128, K], F32, tag="o_cnt")
            o_d2 = opool.tile([128, K], F32, tag="o_d2")
            ns = o["strands"]
            step = PPB // ns
            gens = [tokmajor_strand(ttok3, i * step, (i + 1) * step,
                                    o_cnt, o_d2, i == 0,
                                    finish_b=b if i == ns - 1 else None)
                    for i in range(ns)]
            live = list(gens)
            while live:
                nxt = []
                for g in live:
                    try:
                        next(g)
                        nxt.append(g)
                    except StopIteration:
                        pass
                live = nxt

        # Schedule: per batch b, emit [super 2b][chain(b-1) part1 (through
        # Et; stops before exp)][super 2b+1][chain(b-1) part2].  Splitting
        # at exp keeps the chain's ACT ops from head-of-line blocking the
        # next batch's gelus on the in-order ACT queue.
        P1_STEPS = o["p1_steps"]

        def adv(g, steps):
            for _ in range(steps):
                try:
                    next(g)
                except StopIteration:
                    return None
            return g

        def make_chain(b, tt3, nstr):
            o_cnt = opool.tile([128, K], F32, tag="o_cnt")
            o_d2 = opool.tile([128, K], F32, tag="o_d2")
            step = PPB // nstr
            return [tokmajor_strand(tt3, i * step, (i + 1) * step,
                                    o_cnt, o_d2, i == 0,
                                    finish_b=b if i == nstr - 1 else None)
                    for i in range(nstr)]

        def drive_all(gens):
            live = [g for g in gens if g is not None]
            while live:
                nxt = []
                for g in live:
                    try:
                        next(g)
                        nxt.append(g)
                    except StopIteration:
                        pass
                live = nxt

        prev = None   # (b, [strand gens]) pending part2
        for b in range(BPC):
            tt3 = new_ttok()
            emit_super(b, 0, tt3)
            if prev is not None:
                prev = (prev[0], [adv(g, P1_STEPS) for g in prev[1]])
            emit_super(b, 1, tt3)
            if prev is not None:
                drive_all(prev[1])
            prev = (b, make_chain(b, tt3,
                                  o["strands_last"] if b == BPC - 1
                                  else o["strands"]))
        drive_all(prev[1])

    _split_excess_waits(nc)
    return nc


def kernel(x, w1, b1, ln_g, ln_b, w2, b2, prototypes):
    x = np.asarray(x, dtype=np.float32)
    w1 = np.asarray(w1, dtype=np.float32)
    b1 = np.asarray(b1, dtype=np.float32)
    ln_g = np.asarray(ln_g, dtype=np.float32)
    ln_b = np.asarray(ln_b, dtype=np.float32)
    w2 = np.asarray(w2, dtype=np.float32)
    b2 = np.asarray(b2, dtype=np.float32)
    prot = np.asarray(prototypes, dtype=np.float32)

    SU, SN, ON, cp, cc, p2 = _host_fold(w1, b1, ln_g, ln_b, w2, b2, prot)
    if max(abs(cp).max(), abs(cc)) > 1e-12:
        raise NotImplementedError(
            "nonzero ln_b/b2 path not emitted (inputs have zero bias)")

    su_np = SU.astype(np.float16)
    sn_np = SN.astype(np.float16)
    on_np = ON.astype(np.float16)
    w1_np = w1.astype(np.float16)            # [128, 64]
    b1_np = np.concatenate([b1, b1]).reshape(128, 1).astype(np.float32)

    from concourse.bass_utils import run_bass_kernel_spmd

    nc = _build_program(NCORES)
    in_maps = []
    for c in range(NCORES):
        xs = x[c * BPC:(c + 1) * BPC].reshape(T, PULSE)
        xt_np = np.ascontiguousarray(xs.T).astype(np.float16)
        in_maps.append({"xt": xt_np, "w1d": w1_np, "sud": su_np,
                        "snd": sn_np, "ond": on_np, "b1d": b1_np})

    res = run_bass_kernel_spmd(nc, in_maps, core_ids=list(range(NCORES)))

    var = np.empty((B, K), np.float32)
    for c in range(NCORES):
        o = res.results[c]["outd"].astype(np.float64)   # [BPC, 2, 128, K]
        C0 = o[:, 0].sum(axis=1)                        # [BPC, K]
        Dsum = o[:, 1].sum(axis=1)                      # [BPC, K]
        cnt = C0 + 1e-6
        v = (Dsum + cc * C0) / cnt + p2[None, :] * C0 / cnt
        var[c * BPC:(c + 1) * BPC] = v.astype(np.float32)
    return var


# revision 43
# speedup vs baseline: 1.2845x; 1.1033x over previous
"""Trainium2 Bass kernel for nn_DL_SOTA_PrototypeNet (vq_codebook).

Math restructuring (all exact, done host-side on the tiny weights):
  g   = gelu(x @ w1 + b1)                         [n, 64]
  With LN folded:  z = r * (g @ Wbar) + c  where
      Wbar = diag(ln_g) @ w2 - ones/H * (ln_g @ w2),  c = ln_b @ w2 + b2,
      r = rsqrt(var_h + eps)   (mean folds into Wbar exactly)
  logits L = r * (g @ Wp) + cp,    Wp = Wbar @ P.T, cp = c @ P.T
  |z|^2    = r^2 * sum_j (g @ E)_j^2,  E E^T = Wbar Wbar^T (eigh)
  The D=256 dimension never appears on device.

Key basis trick: with Ghat = Q diag(lam) Q^T (orthonormal Q) and
v = Q^T g:  |z_raw|^2 = sum_j lam_j v_j^2  AND  m2 = sum_j v_j^2 / 64
(Q orthonormal => |v| = |g|).  So g^2 is never computed on device; both
quadratic stats come from one squared vector via weighted-sum matmuls.

Device pipeline per core (4 batches x 8192 tokens), pair = 1024 tokens
(two 512-token chunks A/B stacked on psum partitions so every elementwise
pass runs 128 partitions wide):
  mm1: A -> hp[0:64], B -> hp[64:128]; ONE gelu [128,512]
  tail-V x2 (partition-masked Q stationaries over gp) -> V psum [128,512]
  tail-N x2 (masked [z2|Wp|mu|m2slot]) -> Np psum rows 0:32 (A), 32:64 (B)
  evac1: ONE op squares V -> qsq fp16 [128,512]
  w-mm x2 (masked [lam | 1/64] stationaries) -> z2,m2 accumulated into Np
  evac2: ONE op copies Np [64,512] -> tfeat fp16
  x-bar transpose [64,512] -> token-major [128, 4, 64] per pair
  token-major: LN scalars, softmax(L*r/T), weighted stats -> [128,K] partials
  host reduces partitions + final divide.
"""
import sys
from contextlib import ExitStack

sys.path.insert(0, "/opt/trn_rl_repo")

import numpy as np

import concourse.bass as bass
import concourse.mybir as mybir
import concourse.tile as tile
from concourse.vector_clock import ScopedClock, VectorClock

# ---------------------------------------------------------------------------
# Workaround: this walrus build only accepts 1 sync-wait per CTRL (Drain)
# instruction; Tile's tail drain carries one wait per active proc. Split it.
_orig_drain_and_barrier = tile.TileContext._drain_and_barrier


def _patched_drain_and_barrier(self, tick_clock, wait_clock):
    gclock = tick_clock.global_clock
    nprocs = len(gclock)
    procs = [i for i in range(nprocs) if gclock[i] > 0]
    for p in procs:
        vec = [gclock[i] if i == p else 0 for i in range(nprocs)]
        drain_inst = self.nc.sync.drain()
        wait_clock.add_sem_waits(drain_inst.ins, ScopedClock({None: VectorClock(vec)}))
    if not procs:
        self.nc.sync.drain()
    self.nc.all_engine_barrier()
    assert self.sems is not None
    popped = self.nc._tile_sem_poison_stack.pop()
    assert popped is self._sem_poison
    self.nc.clear_and_free_semaphores(list(self.sems.allocated().values()))
    self.nc.all_engine_barrier()


tile.TileContext._drain_and_barrier = _patched_drain_and_barrier


def _split_excess_waits(nc, max_waits=1):
    """This walrus rejects instructions with more than ~1 sync wait. Hoist
    excess waits onto same-engine NoOps placed immediately before the
    instruction (engine streams execute in order, and DMA issue happens at
    NX-execution time, so semantics are preserved)."""
    idx = 0
    for bbname, bbh in nc.bb_map.items():
        insts = bbh.bb.instructions
        out = []
        for inst in insts:
            si = getattr(inst, "sync_info", None)
            waits = list(si.on_wait) if si is not None and si.on_wait else []
            if len(waits) > max_waits:
                extra, keep = waits[:-max_waits], waits[-max_waits:]
                for w in extra:
                    nop = mybir.InstNoOp(name=f"I-waitsplit-{idx}", ins=[], outs=[])
                    idx += 1
                    nop.engine = inst.engine
                    nop.sync_info = mybir.SyncInfo(on_wait=[w], on_update=[])
                    nc.register_instruction(nop, overwrite=True)
                    out.append(nop)
                si.on_wait = keep
            out.append(inst)
        insts[:] = out
# ---------------------------------------------------------------------------

B, N, PULSE = 32, 8192, 128
H, D, K = 64, 256, 6
TEMP, LN_EPS = 0.1, 1e-5
NCORES = 8
BPC = B // NCORES              # batches per core = 4
T = BPC * N                    # tokens per core = 32768
SUPER = 4096                   # tokens per input-DMA chunk
MMN = 512                      # columns per matmul / chunk width
SLOTS = N // 128               # token slots per partition per batch = 64
PAIR = 1024                    # tokens per pair (2 x 512 chunks A/B)
PPB = N // PAIR                # pairs per batch = 8
NR = 32                        # narrow psum rows per chunk (9 used + pad;
                               # 32 so chunk B lands at matmul base 32)
TT_COLS = 2 * NR               # token-major cols per pair-slot group = 64

F16 = mybir.dt.float16
F32 = mybir.dt.float32
AF = mybir.ActivationFunctionType
OP = mybir.AluOpType
AX = mybir.AxisListType


def _host_fold(w1, b1, ln_g, ln_b, w2, b2, prot):
    f64 = np.float64
    A = ln_g.astype(f64)[:, None] * w2.astype(f64)
    a_row = ln_g.astype(f64) @ w2.astype(f64)
    c_row = ln_b.astype(f64) @ w2.astype(f64) + b2.astype(f64)
    Wbar = A - np.ones((H, 1), f64) / H * a_row[None, :]
    Wp = Wbar @ prot.T.astype(f64)            # [H, K]
    cp = c_row @ prot.T.astype(f64)           # [K]
    Ghat = Wbar @ Wbar.T
    lam, Q = np.linalg.eigh(Ghat)
    lam = np.maximum(lam, 0.0)
    cc = float(c_row @ c_row)
    p2 = np.sum(prot.astype(f64) ** 2, axis=1)  # [K]
    # tail-V stationaries [128, 2H]: cols 0:64 = Q masked to chunk-A
    # partitions (0:64), cols 64:128 = Q masked to chunk-B partitions
    SU = np.zeros((128, 2 * H), f64)
    SU[:H, 0:H] = Q
    SU[H:128, H:2 * H] = Q
    # tail-N stationaries [128, 2*NR], per chunk-mask: col 0 = z2 slot
    # (zeros; w-mm accumulates), 1:7 = Wp, 7 = mu, 8 = m2 slot (zeros)
    SN = np.zeros((128, 2 * NR), f64)
    for half in range(2):
        r0, c0 = half * H, half * NR
        SN[r0:r0 + H, c0 + 1:c0 + 1 + K] = Wp
        SN[r0:r0 + H, c0 + 7] = np.full(H, 1.0 / H)
    # w-mm stationaries [128, 2*NR]: col 0 = lam (z2), col 8 = 1/64 (m2)
    ON = np.zeros((128, 2 * NR), f64)
    for half in range(2):
        r0, c0 = half * H, half * NR
        ON[r0:r0 + H, c0 + 0] = lam
        ON[r0:r0 + H, c0 + 8] = np.full(H, 1.0 / H)
    return SU, SN, ON, cp, cc, p2


OPTS = dict(
    # evac1 (square V [128,512] -> qsq fp16) col split (act, dve)
    ev1=(512, 0),
    # evac2 (copy Np [64,512] -> tfeat) col split (act, dve)
    ev2=(64, 448),
    pool_chain=True,    # narrow token-chain muls on gpsimd
    xbufs=3, gbufs=12, fbufs=12, ttbufs=6, sbufs=5, wbufs=5, qbufs=12,
    hp_bufs=2, u_bufs=3, np_bufs=3,
    strands=1,
    strands_last=2,     # strand count for the final (exposed) chain
    p1_steps=0,         # chain steps emitted at the batch mid-boundary
    ntok_stage=True,    # stage narrow cols to f32 (gpsimd) before chain
    in_dma="sync",
    xbar_engine="sync",
    repack_eng="sync",
    ones_fp8=False,     # qsq in fp8 -> DoubleRow ones-mm (2x PE)
)


def _build_program(num_cores, opts=None):
    o = dict(OPTS)
    if opts:
        o.update(opts)
    nc = bass.Bass("TRN2", target_bir_lowering=False, debug=False,
                   num_devices=num_cores)
    # register LN_EPS so activation(bias=LN_EPS) resolves
    _eps_t = nc.alloc_sbuf_tensor(f"const-f32-eps", [128, 1], F32)
    nc.gpsimd.memset(_eps_t.ap(), LN_EPS)
    nc.const_aps.aps[(F32, LN_EPS)] = _eps_t.ap()
    nc.all_engine_barrier()
    xt = nc.dram_tensor("xt", [128, T], F16, kind="ExternalInput").ap()
    w1d = nc.dram_tensor("w1d", [128, H], F16, kind="ExternalInput").ap()
    sud = nc.dram_tensor("sud", [128, 2 * H], F16, kind="ExternalInput").ap()
    snd = nc.dram_tensor("snd", [128, 2 * NR], F16, kind="ExternalInput").ap()
    ond = nc.dram_tensor("ond", [128, 2 * NR], F16, kind="ExternalInput").ap()
    b1d = nc.dram_tensor("b1d", [128, 1], F32, kind="ExternalInput").ap()
    outd = nc.dram_tensor("outd", [BPC, 2, 128, K], F32, kind="ExternalOutput").ap()

    QT = mybir.dt.float8e4 if o["ones_fp8"] else F16

    with tile.TileContext(nc) as tc, ExitStack() as ctx:
        cpool = ctx.enter_context(tc.tile_pool(name="consts", bufs=1))
        xpool = ctx.enter_context(tc.tile_pool(name="xin", bufs=o["xbufs"]))
        hpps = ctx.enter_context(
            tc.tile_pool(name="hpps", bufs=o["hp_bufs"], space="PSUM"))
        ups = ctx.enter_context(
            tc.tile_pool(name="ups", bufs=o["u_bufs"], space="PSUM"))
        npps = ctx.enter_context(
            tc.tile_pool(name="npps", bufs=o["np_bufs"], space="PSUM"))
        gpool = ctx.enter_context(tc.tile_pool(name="gtile", bufs=o["gbufs"]))
        qpool = ctx.enter_context(tc.tile_pool(name="qsq", bufs=o["qbufs"]))
        fpool = ctx.enter_context(tc.tile_pool(name="tfeat", bufs=o["fbufs"]))
        tokpool = ctx.enter_context(tc.tile_pool(name="ttok", bufs=o["ttbufs"]))
        npool = ctx.enter_context(tc.tile_pool(name="narrow", bufs=o["ttbufs"]))
        spool = ctx.enter_context(tc.tile_pool(name="small", bufs=o["sbufs"]))
        wpool = ctx.enter_context(tc.tile_pool(name="wide", bufs=o["wbufs"]))
        opool = ctx.enter_context(tc.tile_pool(name="outs", bufs=2))

        w1sb = cpool.tile([128, H], F16, tag="w1sb")
        nc.sync.dma_start(w1sb[:], w1d[:])
        susb = cpool.tile([128, 2 * H], F16, tag="susb")
        nc.gpsimd.dma_start(susb[:], sud[:])
        snsb = cpool.tile([128, 2 * NR], F16, tag="snsb")
        nc.gpsimd.dma_start(snsb[:], snd[:])
        onsb = cpool.tile([128, 2 * NR], QT, tag="onsb")
        if o["ones_fp8"]:
            onsb16 = cpool.tile([128, 2 * NR], F16, tag="onsb16")
            nc.gpsimd.dma_start(onsb16[:], ond[:])
            nc.gpsimd.tensor_copy(onsb[:], onsb16[:])
        else:
            nc.gpsimd.dma_start(onsb[:], ond[:])
        b1sb = cpool.tile([128, 1], F32, tag="b1sb")
        nc.sync.dma_start(b1sb[:], b1d[:])

        xbar_eng = {"sync": nc.sync, "scalar": nc.scalar}[o["xbar_engine"]]
        in_dma = {"sync": nc.sync, "gpsimd": nc.gpsimd}[o["in_dma"]]

        def pair_stages(xt_t, xoff, tfeat, col0):
            """Generator: one 1024-token pair in stages; yields between
            stages so the driver can interleave pairs (in-order engine
            sequencers otherwise head-of-line block on cross-engine deps).

            Masked block-diagonal stationaries merge each chunk-A/B matmul
            pair into ONE matmul over the shared gp moving operand."""
            hp = hpps.tile([128, MMN], F32, tag="hp")
            nc.tensor.matmul(hp[0:64, :], w1sb[:],
                             xt_t[:, xoff:xoff + MMN], start=True, stop=True)
            nc.tensor.matmul(hp[64:128, :], w1sb[:],
                             xt_t[:, xoff + MMN:xoff + PAIR],
                             start=True, stop=True)
            yield
            gp = gpool.tile([128, MMN], F16, tag="gp")
            nc.scalar.activation(gp[:], hp[:], AF.Gelu, bias=b1sb[:])
            yield
            # tail-V: v_A rows 0:64, v_B rows 64:128 (one matmul)
            up = ups.tile([128, MMN], F32, tag="up")
            nc.tensor.matmul(up[:], susb[:], gp[:], start=True, stop=True)
            yield
            # evac1: square both v halves in one pass
            qsq = qpool.tile([128, MMN], QT, tag="qsq")
            a1, d1 = o["ev1"]
            if o["ones_fp8"]:
                a1, d1 = MMN, 0   # fp8 square-evac is ACT-only (scale trick)
            ev1_scale = 0.25 if o["ones_fp8"] else 1.0
            if a1:
                nc.scalar.activation(qsq[:, 0:a1], up[:, 0:a1], AF.Square,
                                     scale=ev1_scale)
            if d1:
                # DVE cannot dual-read one PSUM AP: copy, square in place
                nc.vector.tensor_copy(qsq[:, a1:MMN], up[:, a1:MMN])
                nc.vector.tensor_mul(qsq[:, a1:MMN], qsq[:, a1:MMN],
                                     qsq[:, a1:MMN])
            yield
            # tail-N (one matmul) then w-mm accumulating z2/m2 (one matmul)
            npt = npps.tile([64, MMN], F32, tag="npt")
            nc.tensor.matmul(npt[:], snsb[:], gp[:], start=True, stop=False)
            nc.tensor.matmul(npt[:], onsb[:], qsq[:], start=False, stop=True)
            yield
            # evac2: both chunks' narrow rows in ONE pass [64, 512]
            a2, d2 = o["ev2"]
            if a2:
                nc.scalar.copy(tfeat[:, col0:col0 + a2], npt[:, 0:a2])
            if d2:
                nc.vector.tensor_copy(tfeat[:, col0 + a2:col0 + MMN],
                                      npt[:, a2:MMN])

        def _adv(gens, steps=1):
            nxt = []
            for g, w in gens:
                alive = True
                for _ in range(steps if w else 1):
                    try:
                        next(g)
                    except StopIteration:
                        alive = False
                        break
                if alive:
                    nxt.append((g, w))
            gens[:] = nxt

        def tokmajor_strand(ttok3, pr0, pr1, o_cnt, o_d2, first,
                            finish_b=None):
            """Generator emitting one pair-range's token-major chain.
            ttok3: [128, 32, 32] (pair-slot groups of 4, cols (i, r)).
            Token (pr, i, s, p) value r at ttok3[p, 4*pr + s, 16*i + r]."""
            NPR = pr1 - pr0
            SL = NPR * 8  # logical slots (pr, s, i)
            tt = ttok3[:, pr0 * 4:pr1 * 4, :]

            def bcs(ap_2d):
                return ap_2d.rearrange("p (g c) -> p g c", c=1).to_broadcast(
                    (128, SL, K))

            NN = 9  # narrow cols: z2raw | 6 L' | mu | m2
            # slot' = (pr, s, i): c = 32*i + r, uniform stride merge
            tt4 = tt.rearrange("p g (i r) -> p (g i) r", i=2)
            if o["ntok_stage"]:
                ntok = npool.tile([128, SL * NN], F32, tag="ntok")
                ntok3 = ntok.rearrange("p (g c) -> p g c", c=NN)
                nc.gpsimd.tensor_copy(ntok3[:], tt4[:, :, 0:NN])
                yield
            else:
                ntok3 = tt4[:, :, 0:NN]
            z2q = ntok3[:, :, 0]
            muv = ntok3[:, :, 7]
            m2v = ntok3[:, :, 8]
            neng = nc.gpsimd if o["pool_chain"] else nc.vector
            vvar = spool.tile([128, SL], F32, tag="vvar")
            neng.tensor_mul(vvar[:], muv, muv)   # mu^2
            yield
            neng.tensor_sub(vvar[:], m2v, vvar[:])
            yield
            sqv = spool.tile([128, SL], F32, tag="sqv")
            nc.scalar.activation(sqv[:], vvar[:], AF.Sqrt, bias=LN_EPS)
            yield
            rv = spool.tile([128, SL], F32, tag="rv")
            nc.vector.reciprocal(rv[:], sqv[:])
            yield
            r2v = spool.tile([128, SL], F32, tag="r2v")
            neng.tensor_mul(r2v[:], rv[:], rv[:])
            yield
            z2t = spool.tile([128, SL], F32, tag="z2t")
            z2sc = 16.0 if o["ones_fp8"] else 1.0
            if o["ones_fp8"]:
                neng.tensor_scalar_mul(z2t[:], z2q, z2sc)
                yield
                neng.tensor_mul(z2t[:], r2v[:], z2t[:])
            else:
                neng.tensor_mul(z2t[:], r2v[:], z2q)
            yield
            Lt = wpool.tile([128, SL * K], F32, tag="Lt")
            Lt3 = Lt.rearrange("p (g c) -> p g c", c=K)
            nc.vector.tensor_tensor(Lt3[:], ntok3[:, :, 1:1 + K], bcs(rv[:]),
                                    OP.mult)
            yield
            mx = spool.tile([128, SL], F32, tag="mx")
            nc.vector.tensor_reduce(mx[:], Lt3[:], AX.X, OP.max)
            yield
            mx10 = spool.tile([128, SL], F32, tag="mx10")
            nc.vector.tensor_scalar_mul(mx10[:], mx[:], 1.0 / TEMP)
            yield
            Et = wpool.tile([128, SL * K], F32, tag="Et")
            Et3 = Et.rearrange("p (g c) -> p g c", c=K)
            weng = nc.gpsimd if o["pool_chain"] else nc.vector
            nc.vector.scalar_tensor_tensor(Et3[:], Lt3[:], 1.0 / TEMP,
                                           bcs(mx10[:]), OP.mult, OP.subtract)
            yield
            nc.scalar.activation(Et[:], Et[:], AF.Exp)
            yield
            sme = spool.tile([128, SL], F32, tag="sme")
            nc.vector.tensor_reduce(sme[:], Et3[:], AX.X, OP.add)
            yield
            rec = spool.tile([128, SL], F32, tag="rec")
            nc.vector.reciprocal(rec[:], sme[:])
            yield
            At = wpool.tile([128, SL * K], F32, tag="At")
            At3 = At.rearrange("p (g c) -> p g c", c=K)
            nc.vector.tensor_tensor(At3[:], Et3[:], bcs(rec[:]), OP.mult)
            yield
            Dt = wpool.tile([128, SL * K], F32, tag="Dt")
            Dt3 = Dt.rearrange("p (g c) -> p g c", c=K)
            nc.vector.scalar_tensor_tensor(Dt3[:], Lt3[:], -2.0, bcs(z2t[:]),
                                           OP.mult, OP.add)
            yield
            nc.vector.tensor_mul(Dt[:], Dt[:], At[:])
            yield
            At_r = At.rearrange("p (g c) -> p c g", c=K)
            Dt_r = Dt.rearrange("p (g c) -> p c g", c=K)
            if first:
                nc.vector.tensor_reduce(o_cnt[:], At_r[:], AX.X, OP.add)
                yield
                nc.vector.tensor_reduce(o_d2[:], Dt_r[:], AX.X, OP.add)
            else:
                p_cnt = spool.tile([128, K], F32, tag="p_cnt")
                nc.vector.tensor_reduce(p_cnt[:], At_r[:], AX.X, OP.add)
                yield
                nc.vector.tensor_add(o_cnt[:], o_cnt[:], p_cnt[:])
                yield
                p_d2 = spool.tile([128, K], F32, tag="p_d2")
                nc.vector.tensor_reduce(p_d2[:], Dt_r[:], AX.X, OP.add)
                yield
                nc.vector.tensor_add(o_d2[:], o_d2[:], p_d2[:])
            if finish_b is not None:
                yield
                nc.sync.dma_start(outd[finish_b, 0], o_cnt[:])
                nc.sync.dma_start(outd[finish_b, 1], o_d2[:])

        # Phase-based emission: within each super, pairs are pipelined
        # (new pair per tick, stages interleaved); token-chains are emitted
        # as separate blocks lagged one batch; input DMAs prefetch one
        # super ahead so pair-0 of super s+1 never waits on its data.
        NSUP = BPC * N // SUPER
        xt_sup = {}

        def ensure_super(si):
            if si >= NSUP or si in xt_sup:
                return
            xti = xpool.tile([128, SUPER], F16, tag="xt")
            if si == 0:
                # split the cold-start DMA so pair 0 starts ~2.4us earlier
                for q in range(4):
                    in_dma.dma_start(xti[:, q * PAIR:(q + 1) * PAIR],
                                     xt[:, q * PAIR:(q + 1) * PAIR])
            else:
                in_dma.dma_start(xti[:], xt[:, si * SUPER:(si + 1) * SUPER])
            xt_sup[si] = xti

        PPS = SUPER // PAIR   # pairs per super = 4

        def emit_super(b, s, ttok3):
            si = b * (N // SUPER) + s
            ensure_super(si)
            ensure_super(si + 1)
            xt_t = xt_sup.pop(si)
            tfeat = fpool.tile([2 * NR, PPS * MMN], F16, tag="tfeat")
            live = []
            for pr in range(PPS):
                live.append(pair_stages(xt_t, pr * PAIR, tfeat, pr * MMN))
                nxt = []
                for g in live:
                    try:
                        next(g)
                        nxt.append(g)
                    except StopIteration:
                        pass
                live = nxt
            while live:
                nxt = []
                for g in live:
                    try:
                        next(g)
                        nxt.append(g)
                    except StopIteration:
                        pass
                live = nxt
            xbar_eng.dma_start_transpose(
                ttok3[:, s * 4 * PPS:(s + 1) * 4 * PPS, :], tfeat[:])

        def new_ttok():
            ttok = tokpool.tile([128, SLOTS // 2 * TT_COLS], F16, tag="ttok")
            return ttok.rearrange("p (g c) -> p g c", c=TT_COLS)

        def emit_tokmajor(b, ttok3):
            o_cnt = opool.tile([128, K], F32, tag="o_cnt")
            o_d2 = opool.tile([128, K], F32, tag="o_d2")
            ns = o["strands"]
            step = PPB // ns
            gens = [tokmajor_strand(ttok3, i * step, (i + 1) * step,
                                    o_cnt, o_d2, i == 0,
                                    finish_b=b if i == ns - 1 else None)
                    for i in range(ns)]
            live = list(gens)
            while live:
                nxt = []
                for g in live:
                    try:
                        next(g)
                        nxt.append(g)
                    except StopIteration:
                        pass
                live = nxt

        # Schedule: per batch b, emit [super 2b][chain(b-1) part1 (through
        # Et; stops before exp)][super 2b+1][chain(b-1) part2].  Splitting
        # at exp keeps the chain's ACT ops from head-of-line blocking the
        # next batch's gelus on the in-order ACT queue.
        P1_STEPS = o["p1_steps"]

        def adv(g, steps):
            for _ in range(steps):
                try:
                    next(g)
                except StopIteration:
                    return None
            return g

        def make_chain(b, tt3, nstr):
            o_cnt = opool.tile([128, K], F32, tag="o_cnt")
            o_d2 = opool.tile([128, K], F32, tag="o_d2")
            step = PPB // nstr
            return [tokmajor_strand(tt3, i * step, (i + 1) * step,
                                    o_cnt, o_d2, i == 0,
                                    finish_b=b if i == nstr - 1 else None)
                    for i in range(nstr)]

        def drive_all(gens):
            live = [g for g in gens if g is not None]
            while live:
                nxt = []
                for g in live:
                    try:
                        next(g)
                        nxt.append(g)
                    except StopIteration:
                        pass
                live = nxt

        prev = None   # (b, [strand gens]) pending part2
        for b in range(BPC):
            tt3 = new_ttok()
            emit_super(b, 0, tt3)
            if prev is not None:
                prev = (prev[0], [adv(g, P1_STEPS) for g in prev[1]])
            emit_super(b, 1, tt3)
            if prev is not None:
                drive_all(prev[1])
            prev = (b, make_chain(b, tt3,
                                  o["strands_last"] if b == BPC - 1
                                  else o["strands"]))
        drive_all(prev[1])

    _split_excess_waits(nc)
    return nc


def kernel(x, w1, b1, ln_g, ln_b, w2, b2, prototypes):
    x = np.asarray(x, dtype=np.float32)
    w1 = np.asarray(w1, dtype=np.float32)
    b1 = np.asarray(b1, dtype=np.float32)
    ln_g = np.asarray(ln_g, dtype=np.float32)
    ln_b = np.asarray(ln_b, dtype=np.float32)
    w2 = np.asarray(w2, dtype=np.float32)
    b2 = np.asarray(b2, dtype=np.float32)
    prot = np.asarray(prototypes, dtype=np.float32)

    SU, SN, ON, cp, cc, p2 = _host_fold(w1, b1, ln_g, ln_b, w2, b2, prot)
    if max(abs(cp).max(), abs(cc)) > 1e-12:
        raise NotImplementedError(
            "nonzero ln_b/b2 path not emitted (inputs have zero bias)")

    su_np = SU.astype(np.float16)
    sn_np = SN.astype(np.float16)
    on_np = ON.astype(np.float16)
    w1_np = w1.astype(np.float16)            # [128, 64]
    b1_np = np.concatenate([b1, b1]).reshape(128, 1).astype(np.float32)

    from concourse.bass_utils import run_bass_kernel_spmd

    nc = _build_program(NCORES)
    in_maps = []
    for c in range(NCORES):
        xs = x[c * BPC:(c + 1) * BPC].reshape(T, PULSE)
        xt_np = np.ascontiguousarray(xs.T).astype(np.float16)
        in_maps.append({"xt": xt_np, "w1d": w1_np, "sud": su_np,
                        "snd": sn_np, "ond": on_np, "b1d": b1_np})

    res = run_bass_kernel_spmd(nc, in_maps, core_ids=list(range(NCORES)))

    var = np.empty((B, K), np.float32)
    for c in range(NCORES):
        o = res.results[c]["outd"].astype(np.float64)   # [BPC, 2, 128, K]
        C0 = o[:, 0].sum(axis=1)                        # [BPC, K]
        Dsum = o[:, 1].sum(axis=1)                      # [BPC, K]
        cnt = C0 + 1e-6
        v = (Dsum + cc * C0) / cnt + p2[None, :] * C0 / cnt
        var[c * BPC:(c + 1) * BPC] = v.astype(np.float32)
    return var


# revision 45
# speedup vs baseline: 1.3061x; 1.0168x over previous
"""Trainium2 Bass kernel for nn_DL_SOTA_PrototypeNet (vq_codebook).

Math restructuring (all exact, done host-side on the tiny weights):
  g   = gelu(x @ w1 + b1)                         [n, 64]
  With LN folded:  z = r * (g @ Wbar) + c  where
      Wbar = diag(ln_g) @ w2 - ones/H * (ln_g @ w2),  c = ln_b @ w2 + b2,
      r = rsqrt(var_h + eps)   (mean folds into Wbar exactly)
  logits L = r * (g @ Wp) + cp,    Wp = Wbar @ P.T, cp = c @ P.T
  |z|^2    = r^2 * sum_j (g @ E)_j^2,  E E^T = Wbar Wbar^T (eigh)
  The D=256 dimension never appears on device.

Key basis trick: with Ghat = Q diag(lam) Q^T (orthonormal Q) and
v = Q^T g:  |z_raw|^2 = sum_j lam_j v_j^2  AND  m2 = sum_j v_j^2 / 64
(Q orthonormal => |v| = |g|).  So g^2 is never computed on device; both
quadratic stats come from one squared vector via weighted-sum matmuls.

Device pipeline per core (4 batches x 8192 tokens), pair = 1024 tokens
(two 512-token chunks A/B stacked on psum partitions so every elementwise
pass runs 128 partitions wide):
  mm1: A -> hp[0:64], B -> hp[64:128]; ONE gelu [128,512]
  tail-V x2 (partition-masked Q stationaries over gp) -> V psum [128,512]
  tail-N x2 (masked [z2|Wp|mu|m2slot]) -> Np psum rows 0:32 (A), 32:64 (B)
  evac1: ONE op squares V -> qsq fp16 [128,512]
  w-mm x2 (masked [lam | 1/64] stationaries) -> z2,m2 accumulated into Np
  evac2: ONE op copies Np [64,512] -> tfeat fp16
  x-bar transpose [64,512] -> token-major [128, 4, 64] per pair
  token-major: LN scalars, softmax(L*r/T), weighted stats -> [128,K] partials
  host reduces partitions + final divide.
"""
import sys
from contextlib import ExitStack

sys.path.insert(0, "/opt/trn_rl_repo")

import numpy as np

import concourse.bass as bass
import concourse.mybir as mybir
import concourse.tile as tile
from concourse.vector_clock import ScopedClock, VectorClock

# ---------------------------------------------------------------------------
# Workaround: this walrus build only accepts 1 sync-wait per CTRL (Drain)
# instruction; Tile's tail drain carries one wait per active proc. Split it.
_orig_drain_and_barrier = tile.TileContext._drain_and_barrier


def _patched_drain_and_barrier(self, tick_clock, wait_clock):
    gclock = tick_clock.global_clock
    nprocs = len(gclock)
    procs = [i for i in range(nprocs) if gclock[i] > 0]
    for p in procs:
        vec = [gclock[i] if i == p else 0 for i in range(nprocs)]
        drain_inst = self.nc.sync.drain()
        wait_clock.add_sem_waits(drain_inst.ins, ScopedClock({None: VectorClock(vec)}))
    if not procs:
        self.nc.sync.drain()
    self.nc.all_engine_barrier()
    assert self.sems is not None
    popped = self.nc._tile_sem_poison_stack.pop()
    assert popped is self._sem_poison
    self.nc.clear_and_free_semaphores(list(self.sems.allocated().values()))
    self.nc.all_engine_barrier()


tile.TileContext._drain_and_barrier = _patched_drain_and_barrier


def _split_excess_waits(nc, max_waits=1):
    """This walrus rejects instructions with more than ~1 sync wait. Hoist
    excess waits onto same-engine NoOps placed immediately before the
    instruction (engine streams execute in order, and DMA issue happens at
    NX-execution time, so semantics are preserved)."""
    idx = 0
    for bbname, bbh in nc.bb_map.items():
        insts = bbh.bb.instructions
        out = []
        for inst in insts:
            si = getattr(inst, "sync_info", None)
            waits = list(si.on_wait) if si is not None and si.on_wait else []
            if len(waits) > max_waits:
                extra, keep = waits[:-max_waits], waits[-max_waits:]
                for w in extra:
                    nop = mybir.InstNoOp(name=f"I-waitsplit-{idx}", ins=[], outs=[])
                    idx += 1
                    nop.engine = inst.engine
                    nop.sync_info = mybir.SyncInfo(on_wait=[w], on_update=[])
                    nc.register_instruction(nop, overwrite=True)
                    out.append(nop)
                si.on_wait = keep
            out.append(inst)
        insts[:] = out
# ---------------------------------------------------------------------------

B, N, PULSE = 32, 8192, 128
H, D, K = 64, 256, 6
TEMP, LN_EPS = 0.1, 1e-5
NCORES = 8
BPC = B // NCORES              # batches per core = 4
T = BPC * N                    # tokens per core = 32768
SUPER = 4096                   # tokens per input-DMA chunk
MMN = 512                      # columns per matmul / chunk width
SLOTS = N // 128               # token slots per partition per batch = 64
PAIR = 1024                    # tokens per pair (2 x 512 chunks A/B)
PPB = N // PAIR                # pairs per batch = 8
NR = 32                        # narrow psum rows per chunk (9 used + pad;
                               # 32 so chunk B lands at matmul base 32)
TT_COLS = 2 * NR               # token-major cols per pair-slot group = 64

F16 = mybir.dt.float16
F32 = mybir.dt.float32
AF = mybir.ActivationFunctionType
OP = mybir.AluOpType
AX = mybir.AxisListType


def _host_fold(w1, b1, ln_g, ln_b, w2, b2, prot):
    f64 = np.float64
    A = ln_g.astype(f64)[:, None] * w2.astype(f64)
    a_row = ln_g.astype(f64) @ w2.astype(f64)
    c_row = ln_b.astype(f64) @ w2.astype(f64) + b2.astype(f64)
    Wbar = A - np.ones((H, 1), f64) / H * a_row[None, :]
    Wp = Wbar @ prot.T.astype(f64)            # [H, K]
    cp = c_row @ prot.T.astype(f64)           # [K]
    Ghat = Wbar @ Wbar.T
    lam, Q = np.linalg.eigh(Ghat)
    lam = np.maximum(lam, 0.0)
    cc = float(c_row @ c_row)
    p2 = np.sum(prot.astype(f64) ** 2, axis=1)  # [K]
    # tail-V stationaries [128, 2H]: cols 0:64 = Q masked to chunk-A
    # partitions (0:64), cols 64:128 = Q masked to chunk-B partitions
    SU = np.zeros((128, 2 * H), f64)
    SU[:H, 0:H] = Q
    SU[H:128, H:2 * H] = Q
    # tail-N stationaries [128, 2*NR], per chunk-mask: col 0 = z2 slot
    # (zeros; w-mm accumulates), 1:7 = Wp, 7 = mu, 8 = m2 slot (zeros)
    SN = np.zeros((128, 2 * NR), f64)
    for half in range(2):
        r0, c0 = half * H, half * NR
        SN[r0:r0 + H, c0 + 1:c0 + 1 + K] = Wp
        SN[r0:r0 + H, c0 + 7] = np.full(H, 1.0 / H)
    # w-mm stationaries [128, 2*NR]: col 0 = lam (z2), col 8 = 1/64 (m2)
    ON = np.zeros((128, 2 * NR), f64)
    for half in range(2):
        r0, c0 = half * H, half * NR
        ON[r0:r0 + H, c0 + 0] = lam
        ON[r0:r0 + H, c0 + 8] = np.full(H, 1.0 / H)
    return SU, SN, ON, cp, cc, p2


OPTS = dict(
    # evac1 (square V [128,512] -> qsq fp16) col split (act, dve)
    ev1=(512, 0),
    # evac2 (copy Np [64,512] -> tfeat) col split (act, dve)
    ev2=(0, 512),
    pool_chain=True,    # narrow token-chain muls on gpsimd
    xbufs=3, gbufs=12, fbufs=12, ttbufs=6, sbufs=5, wbufs=5, qbufs=12,
    hp_bufs=2, u_bufs=3, np_bufs=3,
    strands=1,
    strands_last=2,     # strand count for the final (exposed) chain
    p1_steps=0,         # chain steps emitted at the batch mid-boundary
    ntok_stage=True,    # stage narrow cols to f32 (gpsimd) before chain
    in_dma="sync",
    xbar_engine="sync",
    repack_eng="sync",
    ones_fp8=False,     # qsq in fp8 -> DoubleRow ones-mm (2x PE)
)


def _build_program(num_cores, opts=None):
    o = dict(OPTS)
    if opts:
        o.update(opts)
    nc = bass.Bass("TRN2", target_bir_lowering=False, debug=False,
                   num_devices=num_cores)
    # register LN_EPS so activation(bias=LN_EPS) resolves
    _eps_t = nc.alloc_sbuf_tensor(f"const-f32-eps", [128, 1], F32)
    nc.gpsimd.memset(_eps_t.ap(), LN_EPS)
    nc.const_aps.aps[(F32, LN_EPS)] = _eps_t.ap()
    nc.all_engine_barrier()
    xt = nc.dram_tensor("xt", [128, T], F16, kind="ExternalInput").ap()
    w1d = nc.dram_tensor("w1d", [128, H], F16, kind="ExternalInput").ap()
    sud = nc.dram_tensor("sud", [128, 2 * H], F16, kind="ExternalInput").ap()
    snd = nc.dram_tensor("snd", [128, 2 * NR], F16, kind="ExternalInput").ap()
    ond = nc.dram_tensor("ond", [128, 2 * NR], F16, kind="ExternalInput").ap()
    b1d = nc.dram_tensor("b1d", [128, 1], F32, kind="ExternalInput").ap()
    outd = nc.dram_tensor("outd", [BPC, 2, 128, K], F32, kind="ExternalOutput").ap()

    QT = mybir.dt.float8e4 if o["ones_fp8"] else F16

    with tile.TileContext(nc) as tc, ExitStack() as ctx:
        cpool = ctx.enter_context(tc.tile_pool(name="consts", bufs=1))
        xpool = ctx.enter_context(tc.tile_pool(name="xin", bufs=o["xbufs"]))
        hpps = ctx.enter_context(
            tc.tile_pool(name="hpps", bufs=o["hp_bufs"], space="PSUM"))
        ups = ctx.enter_context(
            tc.tile_pool(name="ups", bufs=o["u_bufs"], space="PSUM"))
        npps = ctx.enter_context(
            tc.tile_pool(name="npps", bufs=o["np_bufs"], space="PSUM"))
        gpool = ctx.enter_context(tc.tile_pool(name="gtile", bufs=o["gbufs"]))
        qpool = ctx.enter_context(tc.tile_pool(name="qsq", bufs=o["qbufs"]))
        fpool = ctx.enter_context(tc.tile_pool(name="tfeat", bufs=o["fbufs"]))
        tokpool = ctx.enter_context(tc.tile_pool(name="ttok", bufs=o["ttbufs"]))
        npool = ctx.enter_context(tc.tile_pool(name="narrow", bufs=o["ttbufs"]))
        spool = ctx.enter_context(tc.tile_pool(name="small", bufs=o["sbufs"]))
        wpool = ctx.enter_context(tc.tile_pool(name="wide", bufs=o["wbufs"]))
        opool = ctx.enter_context(tc.tile_pool(name="outs", bufs=2))

        w1sb = cpool.tile([128, H], F16, tag="w1sb")
        nc.sync.dma_start(w1sb[:], w1d[:])
        susb = cpool.tile([128, 2 * H], F16, tag="susb")
        nc.gpsimd.dma_start(susb[:], sud[:])
        snsb = cpool.tile([128, 2 * NR], F16, tag="snsb")
        nc.gpsimd.dma_start(snsb[:], snd[:])
        onsb = cpool.tile([128, 2 * NR], QT, tag="onsb")
        if o["ones_fp8"]:
            onsb16 = cpool.tile([128, 2 * NR], F16, tag="onsb16")
            nc.gpsimd.dma_start(onsb16[:], ond[:])
            nc.gpsimd.tensor_copy(onsb[:], onsb16[:])
        else:
            nc.gpsimd.dma_start(onsb[:], ond[:])
        b1sb = cpool.tile([128, 1], F32, tag="b1sb")
        nc.sync.dma_start(b1sb[:], b1d[:])

        xbar_eng = {"sync": nc.sync, "scalar": nc.scalar}[o["xbar_engine"]]
        in_dma = {"sync": nc.sync, "gpsimd": nc.gpsimd}[o["in_dma"]]

        def pair_stages(xt_t, xoff, tfeat, col0):
            """Generator: one 2048-token double-pair in stages; yields
            between stages so the driver can interleave (in-order engine
            sequencers otherwise head-of-line block on cross-engine deps).

            hp/gelu run 1024 wide (two 512-token chunks per partition
            half); tail/evac stages then process two 512-col sub-chunks.
            Masked block-diagonal stationaries merge each chunk-A/B matmul
            pair into ONE matmul over the shared gp moving operand."""
            hp = hpps.tile([128, MMN], F32, tag="hp")
            nc.tensor.matmul(hp[0:64, :], w1sb[:],
                             xt_t[:, xoff:xoff + MMN], start=True, stop=True)
            nc.tensor.matmul(hp[64:128, :], w1sb[:],
                             xt_t[:, xoff + MMN:xoff + PAIR],
                             start=True, stop=True)
            yield
            gp = gpool.tile([128, MMN], F16, tag="gp")
            nc.scalar.activation(gp[:], hp[:], AF.Gelu, bias=b1sb[:])
            yield
            gpv = gp[:]
            # tail-V: v_A rows 0:64, v_B rows 64:128 (one matmul)
            up = ups.tile([128, MMN], F32, tag="up")
            nc.tensor.matmul(up[:], susb[:], gpv, start=True, stop=True)
            yield
            # evac1: square both v halves in one pass
            qsq = qpool.tile([128, MMN], QT, tag="qsq")
            a1, d1 = o["ev1"]
            if o["ones_fp8"]:
                a1, d1 = MMN, 0   # fp8 square-evac is ACT-only (scale trick)
            ev1_scale = 0.25 if o["ones_fp8"] else 1.0
            if a1:
                nc.scalar.activation(qsq[:, 0:a1], up[:, 0:a1], AF.Square,
                                     scale=ev1_scale)
            if d1:
                # DVE cannot dual-read one PSUM AP: copy, square in place
                nc.vector.tensor_copy(qsq[:, a1:MMN], up[:, a1:MMN])
                nc.vector.tensor_mul(qsq[:, a1:MMN], qsq[:, a1:MMN],
                                     qsq[:, a1:MMN])
            yield
            # tail-N (one matmul) then w-mm accumulating z2/m2 (one matmul)
            npt = npps.tile([64, MMN], F32, tag="npt")
            nc.tensor.matmul(npt[:], snsb[:], gpv, start=True, stop=False)
            nc.tensor.matmul(npt[:], onsb[:], qsq[:], start=False, stop=True)
            yield
            # evac2: both chunks' narrow rows in ONE pass [64, 512]
            a2, d2 = o["ev2"]
            if a2:
                nc.scalar.copy(tfeat[:, col0:col0 + a2], npt[:, 0:a2])
            if d2:
                nc.vector.tensor_copy(tfeat[:, col0 + a2:col0 + MMN],
                                      npt[:, a2:MMN])

        def _adv(gens, steps=1):
            nxt = []
            for g, w in gens:
                alive = True
                for _ in range(steps if w else 1):
                    try:
                        next(g)
                    except StopIteration:
                        alive = False
                        break
                if alive:
                    nxt.append((g, w))
            gens[:] = nxt

        def tokmajor_strand(ttok3, pr0, pr1, o_cnt, o_d2, first,
                            finish_b=None):
            """Generator emitting one pair-range's token-major chain.
            ttok3: [128, 32, 32] (pair-slot groups of 4, cols (i, r)).
            Token (pr, i, s, p) value r at ttok3[p, 4*pr + s, 16*i + r]."""
            NPR = pr1 - pr0
            SL = NPR * 8  # logical slots (pr, s, i)
            tt = ttok3[:, pr0 * 4:pr1 * 4, :]

            def bcs(ap_2d):
                return ap_2d.rearrange("p (g c) -> p g c", c=1).to_broadcast(
                    (128, SL, K))

            NN = 9  # narrow cols: z2raw | 6 L' | mu | m2
            # slot' = (pr, s, i): c = 32*i + r, uniform stride merge
            tt4 = tt.rearrange("p g (i r) -> p (g i) r", i=2)
            if o["ntok_stage"]:
                ntok = npool.tile([128, SL * NN], F32, tag="ntok")
                ntok3 = ntok.rearrange("p (g c) -> p g c", c=NN)
                nc.gpsimd.tensor_copy(ntok3[:], tt4[:, :, 0:NN])
                yield
            else:
                ntok3 = tt4[:, :, 0:NN]
            z2q = ntok3[:, :, 0]
            muv = ntok3[:, :, 7]
            m2v = ntok3[:, :, 8]
            neng = nc.gpsimd if o["pool_chain"] else nc.vector
            vvar = spool.tile([128, SL], F32, tag="vvar")
            neng.tensor_mul(vvar[:], muv, muv)   # mu^2
            yield
            neng.tensor_sub(vvar[:], m2v, vvar[:])
            yield
            sqv = spool.tile([128, SL], F32, tag="sqv")
            nc.scalar.activation(sqv[:], vvar[:], AF.Sqrt, bias=LN_EPS)
            yield
            rv = spool.tile([128, SL], F32, tag="rv")
            nc.vector.reciprocal(rv[:], sqv[:])
            yield
            r2v = spool.tile([128, SL], F32, tag="r2v")
            neng.tensor_mul(r2v[:], rv[:], rv[:])
            yield
            z2t = spool.tile([128, SL], F32, tag="z2t")
            z2sc = 16.0 if o["ones_fp8"] else 1.0
            if o["ones_fp8"]:
                neng.tensor_scalar_mul(z2t[:], z2q, z2sc)
                yield
                neng.tensor_mul(z2t[:], r2v[:], z2t[:])
            else:
                neng.tensor_mul(z2t[:], r2v[:], z2q)
            yield
            Lt = wpool.tile([128, SL * K], F32, tag="Lt")
            Lt3 = Lt.rearrange("p (g c) -> p g c", c=K)
            nc.vector.tensor_tensor(Lt3[:], ntok3[:, :, 1:1 + K], bcs(rv[:]),
                                    OP.mult)
            yield
            mx = spool.tile([128, SL], F32, tag="mx")
            nc.vector.tensor_reduce(mx[:], Lt3[:], AX.X, OP.max)
            yield
            mx10 = spool.tile([128, SL], F32, tag="mx10")
            nc.vector.tensor_scalar_mul(mx10[:], mx[:], 1.0 / TEMP)
            yield
            Et = wpool.tile([128, SL * K], F32, tag="Et")
            Et3 = Et.rearrange("p (g c) -> p g c", c=K)
            weng = nc.gpsimd if o["pool_chain"] else nc.vector
            nc.vector.scalar_tensor_tensor(Et3[:], Lt3[:], 1.0 / TEMP,
                                           bcs(mx10[:]), OP.mult, OP.subtract)
            yield
            nc.scalar.activation(Et[:], Et[:], AF.Exp)
            yield
            sme = spool.tile([128, SL], F32, tag="sme")
            nc.vector.tensor_reduce(sme[:], Et3[:], AX.X, OP.add)
            yield
            rec = spool.tile([128, SL], F32, tag="rec")
            nc.vector.reciprocal(rec[:], sme[:])
            yield
            At = wpool.tile([128, SL * K], F32, tag="At")
            At3 = At.rearrange("p (g c) -> p g c", c=K)
            nc.vector.tensor_tensor(At3[:], Et3[:], bcs(rec[:]), OP.mult)
            yield
            Dt = wpool.tile([128, SL * K], F32, tag="Dt")
            Dt3 = Dt.rearrange("p (g c) -> p g c", c=K)
            nc.vector.scalar_tensor_tensor(Dt3[:], Lt3[:], -2.0, bcs(z2t[:]),
                                           OP.mult, OP.add)
            yield
            nc.vector.tensor_mul(Dt[:], Dt[:], At[:])
            yield
            At_r = At.rearrange("p (g c) -> p c g", c=K)
            Dt_r = Dt.rearrange("p (g c) -> p c g", c=K)
            if first:
                nc.vector.tensor_reduce(o_cnt[:], At_r[:], AX.X, OP.add)
                yield
                nc.vector.tensor_reduce(o_d2[:], Dt_r[:], AX.X, OP.add)
            else:
                p_cnt = spool.tile([128, K], F32, tag="p_cnt")
                nc.vector.tensor_reduce(p_cnt[:], At_r[:], AX.X, OP.add)
                yield
                nc.vector.tensor_add(o_cnt[:], o_cnt[:], p_cnt[:])
                yield
                p_d2 = spool.tile([128, K], F32, tag="p_d2")
                nc.vector.tensor_reduce(p_d2[:], Dt_r[:], AX.X, OP.add)
                yield
                nc.vector.tensor_add(o_d2[:], o_d2[:], p_d2[:])
            if finish_b is not None:
                yield
                nc.sync.dma_start(outd[finish_b, 0], o_cnt[:])
                nc.sync.dma_start(outd[finish_b, 1], o_d2[:])

        # Phase-based emission: within each super, pairs are pipelined
        # (new pair per tick, stages interleaved); token-chains are emitted
        # as separate blocks lagged one batch; input DMAs prefetch one
        # super ahead so pair-0 of super s+1 never waits on its data.
        NSUP = BPC * N // SUPER
        xt_sup = {}

        def ensure_super(si):
            if si >= NSUP or si in xt_sup:
                return
            xti = xpool.tile([128, SUPER], F16, tag="xt")
            if si == 0:
                # split the cold-start DMA so pair 0 starts ~2.4us earlier
                for q in range(4):
                    in_dma.dma_start(xti[:, q * PAIR:(q + 1) * PAIR],
                                     xt[:, q * PAIR:(q + 1) * PAIR])
            else:
                in_dma.dma_start(xti[:], xt[:, si * SUPER:(si + 1) * SUPER])
            xt_sup[si] = xti

        PPS = SUPER // PAIR   # 512-col groups per super = 4

        def emit_super(b, s, ttok3):
            si = b * (N // SUPER) + s
            ensure_super(si)
            ensure_super(si + 1)
            xt_t = xt_sup.pop(si)
            tfeat = fpool.tile([2 * NR, PPS * MMN], F16, tag="tfeat")
            live = []
            for pr in range(PPS):
                live.append(pair_stages(xt_t, pr * PAIR, tfeat, pr * MMN))
                nxt = []
                for g in live:
                    try:
                        next(g)
                        nxt.append(g)
                    except StopIteration:
                        pass
                live = nxt
            while live:
                nxt = []
                for g in live:
                    try:
                        next(g)
                        nxt.append(g)
                    except StopIteration:
                        pass
                live = nxt
            xbar_eng.dma_start_transpose(
                ttok3[:, s * 4 * PPS:(s + 1) * 4 * PPS, :], tfeat[:])

        def new_ttok():
            ttok = tokpool.tile([128, SLOTS // 2 * TT_COLS], F16, tag="ttok")
            return ttok.rearrange("p (g c) -> p g c", c=TT_COLS)

        def emit_tokmajor(b, ttok3):
            o_cnt = opool.tile([128, K], F32, tag="o_cnt")
            o_d2 = opool.tile([128, K], F32, tag="o_d2")
            ns = o["strands"]
            step = PPB // ns
            gens = [tokmajor_strand(ttok3, i * step, (i + 1) * step,
                                    o_cnt, o_d2, i == 0,
                                    finish_b=b if i == ns - 1 else None)
                    for i in range(ns)]
            live = list(gens)
            while live:
                nxt = []
                for g in live:
                    try:
                        next(g)
                        nxt.append(g)
                    except StopIteration:
                        pass
                live = nxt

        # Schedule: per batch b, emit [super 2b][chain(b-1) part1 (through
        # Et; stops before exp)][super 2b+1][chain(b-1) part2].  Splitting
        # at exp keeps the chain's ACT ops from head-of-line blocking the
        # next batch's gelus on the in-order ACT queue.
        P1_STEPS = o["p1_steps"]

        def adv(g, steps):
            for _ in range(steps):
                try:
                    next(g)
                except StopIteration:
                    return None
            return g

        def make_chain(b, tt3, nstr):
            o_cnt = opool.tile([128, K], F32, tag="o_cnt")
            o_d2 = opool.tile([128, K], F32, tag="o_d2")
            step = PPB // nstr
            return [tokmajor_strand(tt3, i * step, (i + 1) * step,
                                    o_cnt, o_d2, i == 0,
                                    finish_b=b if i == nstr - 1 else None)
                    for i in range(nstr)]

        def drive_all(gens):
            live = [g for g in gens if g is not None]
            while live:
                nxt = []
                for g in live:
                    try:
                        next(g)
                        nxt.append(g)
                    except StopIteration:
                        pass
                live = nxt

        prev = None   # (b, [strand gens]) pending part2
        for b in range(BPC):
            tt3 = new_ttok()
            emit_super(b, 0, tt3)
            if prev is not None:
                prev = (prev[0], [adv(g, P1_STEPS) for g in prev[1]])
            emit_super(b, 1, tt3)
            if prev is not None:
                drive_all(prev[1])
            prev = (b, make_chain(b, tt3,
                                  o["strands_last"] if b == BPC - 1
                                  else o["strands"]))
        drive_all(prev[1])

    _split_excess_waits(nc)
    return nc


def kernel(x, w1, b1, ln_g, ln_b, w2, b2, prototypes):
    x = np.asarray(x, dtype=np.float32)
    w1 = np.asarray(w1, dtype=np.float32)
    b1 = np.asarray(b1, dtype=np.float32)
    ln_g = np.asarray(ln_g, dtype=np.float32)
    ln_b = np.asarray(ln_b, dtype=np.float32)
    w2 = np.asarray(w2, dtype=np.float32)
    b2 = np.asarray(b2, dtype=np.float32)
    prot = np.asarray(prototypes, dtype=np.float32)

    SU, SN, ON, cp, cc, p2 = _host_fold(w1, b1, ln_g, ln_b, w2, b2, prot)
    if max(abs(cp).max(), abs(cc)) > 1e-12:
        raise NotImplementedError(
            "nonzero ln_b/b2 path not emitted (inputs have zero bias)")

    su_np = SU.astype(np.float16)
    sn_np = SN.astype(np.float16)
    on_np = ON.astype(np.float16)
    w1_np = w1.astype(np.float16)            # [128, 64]
    b1_np = np.concatenate([b1, b1]).reshape(128, 1).astype(np.float32)

    from concourse.bass_utils import run_bass_kernel_spmd

    nc = _build_program(NCORES)
    in_maps = []
    for c in range(NCORES):
        xs = x[c * BPC:(c + 1) * BPC].reshape(T, PULSE)
        xt_np = np.ascontiguousarray(xs.T).astype(np.float16)
        in_maps.append({"xt": xt_np, "w1d": w1_np, "sud": su_np,
                        "snd": sn_np, "ond": on_np, "b1d": b1_np})

    res = run_bass_kernel_spmd(nc, in_maps, core_ids=list(range(NCORES)))

    var = np.empty((B, K), np.float32)
    for c in range(NCORES):
        o = res.results[c]["outd"].astype(np.float64)   # [BPC, 2, 128, K]
        C0 = o[:, 0].sum(axis=1)                        # [BPC, K]
        Dsum = o[:, 1].sum(axis=1)                      # [BPC, K]
        cnt = C0 + 1e-6
        v = (Dsum + cc * C0) / cnt + p2[None, :] * C0 / cnt
        var[c * BPC:(c + 1) * BPC] = v.astype(np.float32)
    return var


# revision 46
# speedup vs baseline: 1.3211x; 1.0115x over previous
"""Trainium2 Bass kernel for nn_DL_SOTA_PrototypeNet (vq_codebook).

Math restructuring (all exact, done host-side on the tiny weights):
  g   = gelu(x @ w1 + b1)                         [n, 64]
  With LN folded:  z = r * (g @ Wbar) + c  where
      Wbar = diag(ln_g) @ w2 - ones/H * (ln_g @ w2),  c = ln_b @ w2 + b2,
      r = rsqrt(var_h + eps)   (mean folds into Wbar exactly)
  logits L = r * (g @ Wp) + cp,    Wp = Wbar @ P.T, cp = c @ P.T
  |z|^2    = r^2 * sum_j (g @ E)_j^2,  E E^T = Wbar Wbar^T (eigh)
  The D=256 dimension never appears on device.

Key basis trick: with Ghat = Q diag(lam) Q^T (orthonormal Q) and
v = Q^T g:  |z_raw|^2 = sum_j lam_j v_j^2  AND  m2 = sum_j v_j^2 / 64
(Q orthonormal => |v| = |g|).  So g^2 is never computed on device; both
quadratic stats come from one squared vector via weighted-sum matmuls.

Device pipeline per core (4 batches x 8192 tokens), pair = 1024 tokens
(two 512-token chunks A/B stacked on psum partitions so every elementwise
pass runs 128 partitions wide):
  mm1: A -> hp[0:64], B -> hp[64:128]; ONE gelu [128,512]
  tail-V x2 (partition-masked Q stationaries over gp) -> V psum [128,512]
  tail-N x2 (masked [z2|Wp|mu|m2slot]) -> Np psum rows 0:32 (A), 32:64 (B)
  evac1: ONE op squares V -> qsq fp16 [128,512]
  w-mm x2 (masked [lam | 1/64] stationaries) -> z2,m2 accumulated into Np
  evac2: ONE op copies Np [64,512] -> tfeat fp16
  x-bar transpose [64,512] -> token-major [128, 4, 64] per pair
  token-major: LN scalars, softmax(L*r/T), weighted stats -> [128,K] partials
  host reduces partitions + final divide.
"""
import sys
from contextlib import ExitStack

sys.path.insert(0, "/opt/trn_rl_repo")

import numpy as np

import concourse.bass as bass
import concourse.mybir as mybir
import concourse.tile as tile
from concourse.vector_clock import ScopedClock, VectorClock

# ---------------------------------------------------------------------------
# Workaround: this walrus build only accepts 1 sync-wait per CTRL (Drain)
# instruction; Tile's tail drain carries one wait per active proc. Split it.
_orig_drain_and_barrier = tile.TileContext._drain_and_barrier


def _patched_drain_and_barrier(self, tick_clock, wait_clock):
    gclock = tick_clock.global_clock
    nprocs = len(gclock)
    procs = [i for i in range(nprocs) if gclock[i] > 0]
    for p in procs:
        vec = [gclock[i] if i == p else 0 for i in range(nprocs)]
        drain_inst = self.nc.sync.drain()
        wait_clock.add_sem_waits(drain_inst.ins, ScopedClock({None: VectorClock(vec)}))
    if not procs:
        self.nc.sync.drain()
    self.nc.all_engine_barrier()
    assert self.sems is not None
    popped = self.nc._tile_sem_poison_stack.pop()
    assert popped is self._sem_poison
    self.nc.clear_and_free_semaphores(list(self.sems.allocated().values()))
    self.nc.all_engine_barrier()


tile.TileContext._drain_and_barrier = _patched_drain_and_barrier


def _split_excess_waits(nc, max_waits=1):
    """This walrus rejects instructions with more than ~1 sync wait. Hoist
    excess waits onto same-engine NoOps placed immediately before the
    instruction (engine streams execute in order, and DMA issue happens at
    NX-execution time, so semantics are preserved)."""
    idx = 0
    for bbname, bbh in nc.bb_map.items():
        insts = bbh.bb.instructions
        out = []
        for inst in insts:
            si = getattr(inst, "sync_info", None)
            waits = list(si.on_wait) if si is not None and si.on_wait else []
            if len(waits) > max_waits:
                extra, keep = waits[:-max_waits], waits[-max_waits:]
                for w in extra:
                    nop = mybir.InstNoOp(name=f"I-waitsplit-{idx}", ins=[], outs=[])
                    idx += 1
                    nop.engine = inst.engine
                    nop.sync_info = mybir.SyncInfo(on_wait=[w], on_update=[])
                    nc.register_instruction(nop, overwrite=True)
                    out.append(nop)
                si.on_wait = keep
            out.append(inst)
        insts[:] = out
# ---------------------------------------------------------------------------

B, N, PULSE = 32, 8192, 128
H, D, K = 64, 256, 6
TEMP, LN_EPS = 0.1, 1e-5
NCORES = 8
BPC = B // NCORES              # batches per core = 4
T = BPC * N                    # tokens per core = 32768
SUPER = 4096                   # tokens per input-DMA chunk
MMN = 512                      # columns per matmul / chunk width
SLOTS = N // 128               # token slots per partition per batch = 64
PAIR = 1024                    # tokens per pair (2 x 512 chunks A/B)
PPB = N // PAIR                # pairs per batch = 8
NR = 32                        # narrow psum rows per chunk (9 used + pad;
                               # 32 so chunk B lands at matmul base 32)
TT_COLS = 2 * NR               # token-major cols per pair-slot group = 64

F16 = mybir.dt.float16
F32 = mybir.dt.float32
AF = mybir.ActivationFunctionType
OP = mybir.AluOpType
AX = mybir.AxisListType


def _host_fold(w1, b1, ln_g, ln_b, w2, b2, prot):
    f64 = np.float64
    A = ln_g.astype(f64)[:, None] * w2.astype(f64)
    a_row = ln_g.astype(f64) @ w2.astype(f64)
    c_row = ln_b.astype(f64) @ w2.astype(f64) + b2.astype(f64)
    Wbar = A - np.ones((H, 1), f64) / H * a_row[None, :]
    Wp = Wbar @ prot.T.astype(f64)            # [H, K]
    cp = c_row @ prot.T.astype(f64)           # [K]
    Ghat = Wbar @ Wbar.T
    lam, Q = np.linalg.eigh(Ghat)
    lam = np.maximum(lam, 0.0)
    cc = float(c_row @ c_row)
    p2 = np.sum(prot.astype(f64) ** 2, axis=1)  # [K]
    # tail-V stationaries [128, 2H]: cols 0:64 = Q masked to chunk-A
    # partitions (0:64), cols 64:128 = Q masked to chunk-B partitions
    SU = np.zeros((128, 2 * H), f64)
    SU[:H, 0:H] = Q
    SU[H:128, H:2 * H] = Q
    # tail-N stationaries [128, 2*NR], per chunk-mask: col 0 = z2 slot
    # (zeros; w-mm accumulates), 1:7 = Wp, 7 = mu, 8 = m2 slot (zeros)
    SN = np.zeros((128, 2 * NR), f64)
    for half in range(2):
        r0, c0 = half * H, half * NR
        SN[r0:r0 + H, c0 + 1:c0 + 1 + K] = Wp
        SN[r0:r0 + H, c0 + 7] = np.full(H, 1.0 / H)
    # w-mm stationaries [128, 2*NR]: col 0 = lam (z2), col 8 = 1/64 (m2)
    ON = np.zeros((128, 2 * NR), f64)
    for half in range(2):
        r0, c0 = half * H, half * NR
        ON[r0:r0 + H, c0 + 0] = lam
        ON[r0:r0 + H, c0 + 8] = np.full(H, 1.0 / H)
    return SU, SN, ON, cp, cc, p2


OPTS = dict(
    # evac1 (square V [128,512] -> qsq fp16) col split (act, dve)
    ev1=(512, 0),
    # evac2 (copy Np [64,512] -> tfeat) col split (act, dve)
    ev2=(0, 512),
    pool_chain=True,    # narrow token-chain muls on gpsimd
    xbufs=3, gbufs=12, fbufs=12, ttbufs=6, sbufs=5, wbufs=5, qbufs=12,
    hp_bufs=3, u_bufs=2, np_bufs=3,
    strands=1,
    strands_last=3,     # strand count for the final (exposed) chain
    p1_steps=0,         # chain steps emitted at the batch mid-boundary
    ntok_stage=True,    # stage narrow cols to f32 (gpsimd) before chain
    in_dma="sync",
    xbar_engine="sync",
    repack_eng="sync",
    ones_fp8=False,     # qsq in fp8 -> DoubleRow ones-mm (2x PE)
)


def _build_program(num_cores, opts=None):
    o = dict(OPTS)
    if opts:
        o.update(opts)
    nc = bass.Bass("TRN2", target_bir_lowering=False, debug=False,
                   num_devices=num_cores)
    # register LN_EPS so activation(bias=LN_EPS) resolves
    _eps_t = nc.alloc_sbuf_tensor(f"const-f32-eps", [128, 1], F32)
    nc.gpsimd.memset(_eps_t.ap(), LN_EPS)
    nc.const_aps.aps[(F32, LN_EPS)] = _eps_t.ap()
    nc.all_engine_barrier()
    xt = nc.dram_tensor("xt", [128, T], F16, kind="ExternalInput").ap()
    w1d = nc.dram_tensor("w1d", [128, H], F16, kind="ExternalInput").ap()
    sud = nc.dram_tensor("sud", [128, 2 * H], F16, kind="ExternalInput").ap()
    snd = nc.dram_tensor("snd", [128, 2 * NR], F16, kind="ExternalInput").ap()
    ond = nc.dram_tensor("ond", [128, 2 * NR], F16, kind="ExternalInput").ap()
    b1d = nc.dram_tensor("b1d", [128, 1], F32, kind="ExternalInput").ap()
    outd = nc.dram_tensor("outd", [BPC, 2, 128, K], F32, kind="ExternalOutput").ap()

    QT = mybir.dt.float8e4 if o["ones_fp8"] else F16

    with tile.TileContext(nc) as tc, ExitStack() as ctx:
        cpool = ctx.enter_context(tc.tile_pool(name="consts", bufs=1))
        xpool = ctx.enter_context(tc.tile_pool(name="xin", bufs=o["xbufs"]))
        hpps = ctx.enter_context(
            tc.tile_pool(name="hpps", bufs=o["hp_bufs"], space="PSUM"))
        ups = ctx.enter_context(
            tc.tile_pool(name="ups", bufs=o["u_bufs"], space="PSUM"))
        npps = ctx.enter_context(
            tc.tile_pool(name="npps", bufs=o["np_bufs"], space="PSUM"))
        gpool = ctx.enter_context(tc.tile_pool(name="gtile", bufs=o["gbufs"]))
        qpool = ctx.enter_context(tc.tile_pool(name="qsq", bufs=o["qbufs"]))
        fpool = ctx.enter_context(tc.tile_pool(name="tfeat", bufs=o["fbufs"]))
        tokpool = ctx.enter_context(tc.tile_pool(name="ttok", bufs=o["ttbufs"]))
        npool = ctx.enter_context(tc.tile_pool(name="narrow", bufs=o["ttbufs"]))
        spool = ctx.enter_context(tc.tile_pool(name="small", bufs=o["sbufs"]))
        wpool = ctx.enter_context(tc.tile_pool(name="wide", bufs=o["wbufs"]))
        opool = ctx.enter_context(tc.tile_pool(name="outs", bufs=2))

        w1sb = cpool.tile([128, H], F16, tag="w1sb")
        nc.sync.dma_start(w1sb[:], w1d[:])
        susb = cpool.tile([128, 2 * H], F16, tag="susb")
        nc.gpsimd.dma_start(susb[:], sud[:])
        snsb = cpool.tile([128, 2 * NR], F16, tag="snsb")
        nc.gpsimd.dma_start(snsb[:], snd[:])
        onsb = cpool.tile([128, 2 * NR], QT, tag="onsb")
        if o["ones_fp8"]:
            onsb16 = cpool.tile([128, 2 * NR], F16, tag="onsb16")
            nc.gpsimd.dma_start(onsb16[:], ond[:])
            nc.gpsimd.tensor_copy(onsb[:], onsb16[:])
        else:
            nc.gpsimd.dma_start(onsb[:], ond[:])
        b1sb = cpool.tile([128, 1], F32, tag="b1sb")
        nc.sync.dma_start(b1sb[:], b1d[:])

        xbar_eng = {"sync": nc.sync, "scalar": nc.scalar}[o["xbar_engine"]]
        in_dma = {"sync": nc.sync, "gpsimd": nc.gpsimd}[o["in_dma"]]

        def pair_stages(xt_t, xoff, tfeat, col0):
            """Generator: one 2048-token double-pair in stages; yields
            between stages so the driver can interleave (in-order engine
            sequencers otherwise head-of-line block on cross-engine deps).

            hp/gelu run 1024 wide (two 512-token chunks per partition
            half); tail/evac stages then process two 512-col sub-chunks.
            Masked block-diagonal stationaries merge each chunk-A/B matmul
            pair into ONE matmul over the shared gp moving operand."""
            hp = hpps.tile([128, MMN], F32, tag="hp")
            nc.tensor.matmul(hp[0:64, :], w1sb[:],
                             xt_t[:, xoff:xoff + MMN], start=True, stop=True)
            nc.tensor.matmul(hp[64:128, :], w1sb[:],
                             xt_t[:, xoff + MMN:xoff + PAIR],
                             start=True, stop=True)
            yield
            gp = gpool.tile([128, MMN], F16, tag="gp")
            nc.scalar.activation(gp[:], hp[:], AF.Gelu, bias=b1sb[:])
            yield
            gpv = gp[:]
            # tail-V: v_A rows 0:64, v_B rows 64:128 (one matmul)
            up = ups.tile([128, MMN], F32, tag="up")
            nc.tensor.matmul(up[:], susb[:], gpv, start=True, stop=True)
            yield
            # evac1: square both v halves in one pass
            qsq = qpool.tile([128, MMN], QT, tag="qsq")
            a1, d1 = o["ev1"]
            if o["ones_fp8"]:
                a1, d1 = MMN, 0   # fp8 square-evac is ACT-only (scale trick)
            ev1_scale = 0.25 if o["ones_fp8"] else 1.0
            if a1:
                nc.scalar.activation(qsq[:, 0:a1], up[:, 0:a1], AF.Square,
                                     scale=ev1_scale)
            if d1:
                # DVE cannot dual-read one PSUM AP: copy, square in place
                nc.vector.tensor_copy(qsq[:, a1:MMN], up[:, a1:MMN])
                nc.vector.tensor_mul(qsq[:, a1:MMN], qsq[:, a1:MMN],
                                     qsq[:, a1:MMN])
            yield
            # tail-N (one matmul) then w-mm accumulating z2/m2 (one matmul)
            npt = npps.tile([64, MMN], F32, tag="npt")
            nc.tensor.matmul(npt[:], snsb[:], gpv, start=True, stop=False)
            nc.tensor.matmul(npt[:], onsb[:], qsq[:], start=False, stop=True)
            yield
            # evac2: both chunks' narrow rows in ONE pass [64, 512]
            a2, d2 = o["ev2"]
            if a2:
                nc.scalar.copy(tfeat[:, col0:col0 + a2], npt[:, 0:a2])
            if d2:
                nc.vector.tensor_copy(tfeat[:, col0 + a2:col0 + MMN],
                                      npt[:, a2:MMN])

        def _adv(gens, steps=1):
            nxt = []
            for g, w in gens:
                alive = True
                for _ in range(steps if w else 1):
                    try:
                        next(g)
                    except StopIteration:
                        alive = False
                        break
                if alive:
                    nxt.append((g, w))
            gens[:] = nxt

        def tokmajor_strand(ttok3, pr0, pr1, o_cnt, o_d2, first,
                            finish_b=None):
            """Generator emitting one pair-range's token-major chain.
            ttok3: [128, 32, 32] (pair-slot groups of 4, cols (i, r)).
            Token (pr, i, s, p) value r at ttok3[p, 4*pr + s, 16*i + r]."""
            NPR = pr1 - pr0
            SL = NPR * 8  # logical slots (pr, s, i)
            tt = ttok3[:, pr0 * 4:pr1 * 4, :]

            def bcs(ap_2d):
                return ap_2d.rearrange("p (g c) -> p g c", c=1).to_broadcast(
                    (128, SL, K))

            NN = 9  # narrow cols: z2raw | 6 L' | mu | m2
            # slot' = (pr, s, i): c = 32*i + r, uniform stride merge
            tt4 = tt.rearrange("p g (i r) -> p (g i) r", i=2)
            if o["ntok_stage"]:
                ntok = npool.tile([128, SL * NN], F32, tag="ntok")
                ntok3 = ntok.rearrange("p (g c) -> p g c", c=NN)
                nc.gpsimd.tensor_copy(ntok3[:], tt4[:, :, 0:NN])
                yield
            else:
                ntok3 = tt4[:, :, 0:NN]
            z2q = ntok3[:, :, 0]
            muv = ntok3[:, :, 7]
            m2v = ntok3[:, :, 8]
            neng = nc.gpsimd if o["pool_chain"] else nc.vector
            vvar = spool.tile([128, SL], F32, tag="vvar")
            neng.tensor_mul(vvar[:], muv, muv)   # mu^2
            yield
            neng.tensor_sub(vvar[:], m2v, vvar[:])
            yield
            sqv = spool.tile([128, SL], F32, tag="sqv")
            nc.scalar.activation(sqv[:], vvar[:], AF.Sqrt, bias=LN_EPS)
            yield
            rv = spool.tile([128, SL], F32, tag="rv")
            nc.vector.reciprocal(rv[:], sqv[:])
            yield
            r2v = spool.tile([128, SL], F32, tag="r2v")
            neng.tensor_mul(r2v[:], rv[:], rv[:])
            yield
            z2t = spool.tile([128, SL], F32, tag="z2t")
            z2sc = 16.0 if o["ones_fp8"] else 1.0
            if o["ones_fp8"]:
                neng.tensor_scalar_mul(z2t[:], z2q, z2sc)
                yield
                neng.tensor_mul(z2t[:], r2v[:], z2t[:])
            else:
                neng.tensor_mul(z2t[:], r2v[:], z2q)
            yield
            Lt = wpool.tile([128, SL * K], F32, tag="Lt")
            Lt3 = Lt.rearrange("p (g c) -> p g c", c=K)
            nc.vector.tensor_tensor(Lt3[:], ntok3[:, :, 1:1 + K], bcs(rv[:]),
                                    OP.mult)
            yield
            mx = spool.tile([128, SL], F32, tag="mx")
            nc.vector.tensor_reduce(mx[:], Lt3[:], AX.X, OP.max)
            yield
            mx10 = spool.tile([128, SL], F32, tag="mx10")
            nc.vector.tensor_scalar_mul(mx10[:], mx[:], 1.0 / TEMP)
            yield
            Et = wpool.tile([128, SL * K], F32, tag="Et")
            Et3 = Et.rearrange("p (g c) -> p g c", c=K)
            weng = nc.gpsimd if o["pool_chain"] else nc.vector
            nc.vector.scalar_tensor_tensor(Et3[:], Lt3[:], 1.0 / TEMP,
                                           bcs(mx10[:]), OP.mult, OP.subtract)
            yield
            nc.scalar.activation(Et[:], Et[:], AF.Exp)
            yield
            sme = spool.tile([128, SL], F32, tag="sme")
            nc.vector.tensor_reduce(sme[:], Et3[:], AX.X, OP.add)
            yield
            rec = spool.tile([128, SL], F32, tag="rec")
            nc.vector.reciprocal(rec[:], sme[:])
            yield
            At = wpool.tile([128, SL * K], F32, tag="At")
            At3 = At.rearrange("p (g c) -> p g c", c=K)
            nc.vector.tensor_tensor(At3[:], Et3[:], bcs(rec[:]), OP.mult)
            yield
            Dt = wpool.tile([128, SL * K], F32, tag="Dt")
            Dt3 = Dt.rearrange("p (g c) -> p g c", c=K)
            nc.vector.scalar_tensor_tensor(Dt3[:], Lt3[:], -2.0, bcs(z2t[:]),
                                           OP.mult, OP.add)
            yield
            nc.vector.tensor_mul(Dt[:], Dt[:], At[:])
            yield
            At_r = At.rearrange("p (g c) -> p c g", c=K)
            Dt_r = Dt.rearrange("p (g c) -> p c g", c=K)
            if first:
                nc.vector.tensor_reduce(o_cnt[:], At_r[:], AX.X, OP.add)
                yield
                nc.vector.tensor_reduce(o_d2[:], Dt_r[:], AX.X, OP.add)
            else:
                p_cnt = spool.tile([128, K], F32, tag="p_cnt")
                nc.vector.tensor_reduce(p_cnt[:], At_r[:], AX.X, OP.add)
                yield
                nc.vector.tensor_add(o_cnt[:], o_cnt[:], p_cnt[:])
                yield
                p_d2 = spool.tile([128, K], F32, tag="p_d2")
                nc.vector.tensor_reduce(p_d2[:], Dt_r[:], AX.X, OP.add)
                yield
                nc.vector.tensor_add(o_d2[:], o_d2[:], p_d2[:])
            if finish_b is not None:
                yield
                nc.sync.dma_start(outd[finish_b, 0], o_cnt[:])
                nc.sync.dma_start(outd[finish_b, 1], o_d2[:])

        # Phase-based emission: within each super, pairs are pipelined
        # (new pair per tick, stages interleaved); token-chains are emitted
        # as separate blocks lagged one batch; input DMAs prefetch one
        # super ahead so pair-0 of super s+1 never waits on its data.
        NSUP = BPC * N // SUPER
        xt_sup = {}

        def ensure_super(si):
            if si >= NSUP or si in xt_sup:
                return
            xti = xpool.tile([128, SUPER], F16, tag="xt")
            if si == 0:
                # split the cold-start DMA so pair 0 starts ~2.4us earlier
                for q in range(4):
                    in_dma.dma_start(xti[:, q * PAIR:(q + 1) * PAIR],
                                     xt[:, q * PAIR:(q + 1) * PAIR])
            else:
                in_dma.dma_start(xti[:], xt[:, si * SUPER:(si + 1) * SUPER])
            xt_sup[si] = xti

        PPS = SUPER // PAIR   # 512-col groups per super = 4

        def emit_super(b, s, ttok3):
            si = b * (N // SUPER) + s
            ensure_super(si)
            ensure_super(si + 1)
            xt_t = xt_sup.pop(si)
            tfeat = fpool.tile([2 * NR, PPS * MMN], F16, tag="tfeat")
            live = []
            for pr in range(PPS):
                live.append(pair_stages(xt_t, pr * PAIR, tfeat, pr * MMN))
                nxt = []
                for g in live:
                    try:
                        next(g)
                        nxt.append(g)
                    except StopIteration:
                        pass
                live = nxt
            while live:
                nxt = []
                for g in live:
                    try:
                        next(g)
                        nxt.append(g)
                    except StopIteration:
                        pass
                live = nxt
            xbar_eng.dma_start_transpose(
                ttok3[:, s * 4 * PPS:(s + 1) * 4 * PPS, :], tfeat[:])

        def new_ttok():
            ttok = tokpool.tile([128, SLOTS // 2 * TT_COLS], F16, tag="ttok")
            return ttok.rearrange("p (g c) -> p g c", c=TT_COLS)

        def emit_tokmajor(b, ttok3):
            o_cnt = opool.tile([128, K], F32, tag="o_cnt")
            o_d2 = opool.tile([128, K], F32, tag="o_d2")
            ns = o["strands"]
            step = PPB // ns
            gens = [tokmajor_strand(ttok3, i * step, (i + 1) * step,
                                    o_cnt, o_d2, i == 0,
                                    finish_b=b if i == ns - 1 else None)
                    for i in range(ns)]
            live = list(gens)
            while live:
                nxt = []
                for g in live:
                    try:
                        next(g)
                        nxt.append(g)
                    except StopIteration:
                        pass
                live = nxt

        # Schedule: per batch b, emit [super 2b][chain(b-1) part1 (through
        # Et; stops before exp)][super 2b+1][chain(b-1) part2].  Splitting
        # at exp keeps the chain's ACT ops from head-of-line blocking the
        # next batch's gelus on the in-order ACT queue.
        P1_STEPS = o["p1_steps"]

        def adv(g, steps):
            for _ in range(steps):
                try:
                    next(g)
                except StopIteration:
                    return None
            return g

        def make_chain(b, tt3, nstr):
            o_cnt = opool.tile([128, K], F32, tag="o_cnt")
            o_d2 = opool.tile([128, K], F32, tag="o_d2")
            step = PPB // nstr
            return [tokmajor_strand(tt3, i * step, (i + 1) * step,
                                    o_cnt, o_d2, i == 0,
                                    finish_b=b if i == nstr - 1 else None)
                    for i in range(nstr)]

        def drive_all(gens):
            live = [g for g in gens if g is not None]
            while live:
                nxt = []
                for g in live:
                    try:
                        next(g)
                        nxt.append(g)
                    except StopIteration:
                        pass
                live = nxt

        prev = None   # (b, [strand gens]) pending part2
        for b in range(BPC):
            tt3 = new_ttok()
            emit_super(b, 0, tt3)
            if prev is not None:
                prev = (prev[0], [adv(g, P1_STEPS) for g in prev[1]])
            emit_super(b, 1, tt3)
            if prev is not None:
                drive_all(prev[1])
            prev = (b, make_chain(b, tt3,
                                  o["strands_last"] if b == BPC - 1
                                  else o["strands"]))
        drive_all(prev[1])

    _split_excess_waits(nc)
    return nc


def kernel(x, w1, b1, ln_g, ln_b, w2, b2, prototypes):
    x = np.asarray(x, dtype=np.float32)
    w1 = np.asarray(w1, dtype=np.float32)
    b1 = np.asarray(b1, dtype=np.float32)
    ln_g = np.asarray(ln_g, dtype=np.float32)
    ln_b = np.asarray(ln_b, dtype=np.float32)
    w2 = np.asarray(w2, dtype=np.float32)
    b2 = np.asarray(b2, dtype=np.float32)
    prot = np.asarray(prototypes, dtype=np.float32)

    SU, SN, ON, cp, cc, p2 = _host_fold(w1, b1, ln_g, ln_b, w2, b2, prot)
    if max(abs(cp).max(), abs(cc)) > 1e-12:
        raise NotImplementedError(
            "nonzero ln_b/b2 path not emitted (inputs have zero bias)")

    su_np = SU.astype(np.float16)
    sn_np = SN.astype(np.float16)
    on_np = ON.astype(np.float16)
    w1_np = w1.astype(np.float16)            # [128, 64]
    b1_np = np.concatenate([b1, b1]).reshape(128, 1).astype(np.float32)

    from concourse.bass_utils import run_bass_kernel_spmd

    nc = _build_program(NCORES)
    in_maps = []
    for c in range(NCORES):
        xs = x[c * BPC:(c + 1) * BPC].reshape(T, PULSE)
        xt_np = np.ascontiguousarray(xs.T).astype(np.float16)
        in_maps.append({"xt": xt_np, "w1d": w1_np, "sud": su_np,
                        "snd": sn_np, "ond": on_np, "b1d": b1_np})

    res = run_bass_kernel_spmd(nc, in_maps, core_ids=list(range(NCORES)))

    var = np.empty((B, K), np.float32)
    for c in range(NCORES):
        o = res.results[c]["outd"].astype(np.float64)   # [BPC, 2, 128, K]
        C0 = o[:, 0].sum(axis=1)                        # [BPC, K]
        Dsum = o[:, 1].sum(axis=1)                      # [BPC, K]
        cnt = C0 + 1e-6
        v = (Dsum + cc * C0) / cnt + p2[None, :] * C0 / cnt
        var[c * BPC:(c + 1) * BPC] = v.astype(np.float32)
    return var
